# revision 1
# baseline (speedup 1.0000x reference)
"""MQA attention (B=2, Lq=Lkv=2048, F=1024, H=16, D=64) on 8 TRN2 cores.

Sharding: core = (batch, query-block-of-512). Each core computes its full
output rows (all 16 heads + output projection) -> no collectives; host
concatenates per-core yT slabs.

Per-core dataflow (matmuls in f32r = fp32 rounded to 11-bit mantissa, full
PE rate; only input-rounding error ~1e-4):
  qT[hd,lq]    = Wq'-chunks.T @ xqT          (transposed-producing projections)
  kvT[kd|vd,lk]= Wkv.T @ xkvT
  RoPE applied in a "halves-permuted" head-dim basis (host permutes Wq/Wk
  columns so even dims come first): q_rot = q*cos + Swap @ (q*sin_signed)
  where Swap is a 128x128 permutation matrix applied on the PE.
  S^T[lk,lq] per head = k-chunk.T @ qT      (zero-padded K=128 stationary)
  P^T = exp(S^T) * maskT   (ACT exp on [128,1024] 2-bank PSUM supertiles;
  mask multiply split across DVE and GpSimd)
  O_aug^T = V_aug-chunk.T @ P^T  (ones column in V_aug -> row 64 = softmax
  denominator; reciprocal row broadcast via a K=1 ones matmul on the PE)
  yT[f,lq] = Wo-chunks.T @ obig (+bo)
"""

import ml_dtypes
import numpy as np

import concourse.bass as bass
import concourse.tile as tile
from concourse import bacc, mybir
from concourse import bass_utils
from concourse.bass import ts
from concourse.masks import make_identity

F32 = mybir.dt.float32
F32R = mybir.dt.float32r
BF16 = mybir.dt.bfloat16

B, L, F, H, D = 2, 2048, 1024, 16, 64
LQ = 512            # query rows per core
LK = 2048           # kv rows (full)
NCORES = 8
PAIRS = H // 2      # head pairs (one qT partition block each)
FCH = F // 128      # f contraction chunks
KCH = LK // 128     # lk chunks
NL = LK // LQ       # kv column blocks

_CACHED = {}


def round_f32r(x: np.ndarray) -> np.ndarray:
    """Round-to-nearest-even fp32 -> fp32r (11-bit stored mantissa)."""
    u = np.ascontiguousarray(x, dtype=np.float32).view(np.uint32)
    lsb = (u >> np.uint32(12)) & np.uint32(1)
    u2 = (u + np.uint32(0x7FF) + lsb) & np.uint32(0xFFFFF000)
    return u2.view(np.float32)


def build_nc(debug=False, n_loop=1):
    nc = bacc.Bacc("TRN2", target_bir_lowering=False, debug=False,
                   num_devices=NCORES)
    dt_in = [
        ("xq_t", [FCH, 128, LQ], F32R),        # [f, p, lq]
        ("xkv_t", [NL, FCH, 128, LQ], F32R),   # [l, f, p, lq]
        ("mask_t", [KCH, 128, LQ], mybir.dt.float16),      # [c, p, lq]
        ("wq", [FCH, 128, FCH, 128], F32R),    # [j, p, f, m]
        ("wkv", [128, FCH, 128], F32R),        # [p, f, m]
        ("wo", [FCH, 128, FCH, 128], F32R),    # [fb, p, j, m]
        ("bqbo", [128, 2 * FCH], F32),         # cols 0:8 bq-blocks, 8:16 bo
        ("bkv", [2 * D], F32),
        ("cosq", [128, LQ], F32),
        ("sinq", [128, LQ], F32),
        ("cksk", [D, 2 * LK], F32),            # [p, (cos|sin)*lk]
    ]
    t = {name: nc.dram_tensor(name, shape, dt, kind="ExternalInput")
         for name, shape, dt in dt_in}
    yT = nc.dram_tensor("yT", [F, LQ], F32, kind="ExternalOutput")
    dbg = {}
    if debug:
        for name, shape in [("d_qrot", [128, PAIRS, LQ]), ("d_ktop", [128, LK]),
                            ("d_kbot", [128, LK]), ("d_vaug", [128, KCH, D + 1]),
                            ("d_obig", [128, PAIRS, LQ]),
                            ("d_qraw", [128, FCH, LQ]), ("d_kvraw", [128, LK]),
                            ("d_pt0", [128, 2, LQ])]:
            dbg[name] = nc.dram_tensor(name, shape, F32, kind="ExternalOutput")

    import contextlib
    with tile.TileContext(nc) as tc:
        loop_cm = tc.For_i(0, n_loop, 1) if n_loop > 1 else contextlib.nullcontext()
        with loop_cm:
          with (
              tc.tile_pool(name="persist", bufs=1) as persist,
              tc.tile_pool(name="ptiles", bufs=3) as ptp,
              tc.tile_pool(name="small", bufs=4) as small,
              tc.tile_pool(name="psacc", bufs=4, space="PSUM") as psacc,
              tc.tile_pool(name="psst", bufs=2, space="PSUM") as psst,
          ):
              mt = persist.tile([128, KCH, LQ], mybir.dt.float16)       # maskT resident
              for c in range(KCH):
                  nc.gpsimd.dma_start(mt[:, c, :], t["mask_t"].ap()[c])

              qrot = persist.tile([128, PAIRS, LQ], F32R)   # rotated qT
              ktop = persist.tile([128, LK], F32R)          # k in rows 0:64
              kbot = persist.tile([128, LK], F32R)          # k in rows 64:128
              vaug = persist.tile([128, KCH, D + 1], mybir.dt.float16)  # V chunks + ones col
              obig = persist.tile([128, PAIRS, LQ], F32R)   # normalized O^T

              cq = persist.tile([128, LQ], F32)
              sq = persist.tile([128, LQ], F32)
              cksk = persist.tile([D, 2, LK], F32)
              nc.sync.dma_start(cq, t["cosq"].ap())
              nc.sync.dma_start(sq, t["sinq"].ap())
              nc.sync.dma_start(cksk,
                                t["cksk"].ap().rearrange("p (a l) -> p a l", a=2))
              ck = cksk[:, 0, :]
              sk = cksk[:, 1, :]

              bqbo = small.tile([128, 2 * FCH], F32, tag="bias")
              nc.sync.dma_start(bqbo, t["bqbo"].ap())
              bq_sb = bqbo[:, 0:FCH]
              bo_sb = bqbo[:, FCH:2 * FCH]
              bkv_sb = small.tile([128, 1], F32, tag="bias2")
              nc.sync.dma_start(bkv_sb, t["bkv"].ap().unsqueeze(1))

              idt = small.tile([128, 128], F32, tag="ident")
              make_identity(nc, idt)
              # halves-swap permutation matrix: M[p, p-xor-32-within-head] = 1
              swpf = small.tile([128, 128], F32, tag="swpf")
              nc.gpsimd.memset(swpf, 0.0)
              for o1, o2 in ((0, 32), (32, 0), (64, 96), (96, 64)):
                  nc.gpsimd.affine_select(
                      out=swpf[o1:o1 + 32, o2:o2 + 32],
                      in_=swpf[o1:o1 + 32, o2:o2 + 32],
                      compare_op=mybir.AluOpType.not_equal, fill=1.0,
                      base=0, pattern=[[-1, 32]], channel_multiplier=1)
              swp = small.tile([128, 128], F32R, tag="swp")
              nc.vector.tensor_copy(swp, swpf)
              ones64 = small.tile([128, D], F32R, tag="ones")
              nc.vector.memset(ones64.bitcast(F32), 1.0)

              # ================= phase A/B: projections + RoPE ================
              with (
                  tc.tile_pool(name="xin", bufs=2) as xin,
                  tc.tile_pool(name="wst", bufs=2) as wst,
                  tc.tile_pool(name="qraw", bufs=1) as qrp,
                  tc.tile_pool(name="kvraw", bufs=1) as kvp,
                  tc.tile_pool(name="ropetmp", bufs=2) as rtp,
                  tc.tile_pool(name="ktmp", bufs=1) as ktp,
              ):
                  # ---- q projection ----
                  xq = xin.tile([128, FCH, LQ], F32R, tag="x")
                  for f in range(FCH):
                      nc.sync.dma_start(xq[:, f, :], t["xq_t"].ap()[f])
                  qraw = qrp.tile([128, FCH, LQ], F32)
                  for j in range(FCH):
                      wq_j = wst.tile([128, FCH, 128], F32R, tag="w")
                      nc.sync.dma_start(wq_j, t["wq"].ap()[j])
                      psq = psacc.tile([128, LQ], F32, tag="acc")
                      for f in range(FCH):
                          nc.tensor.matmul(psq, wq_j[:, f, :], xq[:, f, :],
                                           start=(f == 0), stop=(f == FCH - 1))
                      nc.vector.tensor_scalar_add(qraw[:, j, :], psq,
                                                  bq_sb[:, j:j + 1])

                  # ---- kv projection ----
                  wkv_sb = wst.tile([128, FCH, 128], F32R, tag="w")
                  nc.sync.dma_start(wkv_sb, t["wkv"].ap())
                  kvraw = kvp.tile([128, LK], F32)
                  for l in range(NL):
                      xkv = xin.tile([128, FCH, LQ], F32R, tag="x")
                      for f in range(FCH):
                          nc.sync.dma_start(xkv[:, f, :],
                                            t["xkv_t"].ap()[l, f])
                      pkv = psacc.tile([128, LQ], F32, tag="acc")
                      for f in range(FCH):
                          nc.tensor.matmul(pkv, wkv_sb[:, f, :], xkv[:, f, :],
                                           start=(f == 0), stop=(f == FCH - 1))
                      nc.vector.tensor_scalar_add(kvraw[:, ts(l, LQ)], pkv,
                                                  bkv_sb[:, 0:1])

                  # ---- RoPE on q: qrot = q*cos + Swap @ (q*sin_signed) ----
                  for j in range(FCH):
                      tmq = rtp.tile([128, LQ], F32R, tag="qsin")
                      nc.vector.tensor_mul(tmq, qraw[:, j, :], sq)
                      psw = psacc.tile([128, LQ], F32, tag="acc")
                      nc.tensor.matmul(psw, swp, tmq, start=True, stop=True)
                      qc = rtp.tile([128, LQ], F32, tag="qcos")
                      nc.vector.tensor_mul(qc, qraw[:, j, :], cq)
                      nc.vector.tensor_add(qrot[:, j, :], qc, psw)

                  # ---- RoPE on k: matmul-swap; kbot copy via DMA ----
                  tmk = ktp.tile([D, LK], F32R, tag="ksin")
                  nc.vector.tensor_mul(tmk, kvraw[0:64], sk)
                  kc = ktp.tile([D, LK], F32, tag="kcos")
                  nc.vector.tensor_mul(kc, kvraw[0:64], ck)
                  nc.vector.memset(ktop[64:128].bitcast(F32), 0.0)
                  nc.vector.memset(kbot[0:64].bitcast(F32), 0.0)
                  for l in range(NL):
                      pswk = psacc.tile([128, LQ], F32, tag="acc")
                      nc.tensor.matmul(pswk[0:64], swp[0:64, 0:64],
                                       tmk[:, ts(l, LQ)], start=True, stop=True)
                      nc.vector.tensor_add(ktop[0:64, ts(l, LQ)],
                                           kc[:, ts(l, LQ)], pswk[0:64])
                  nc.gpsimd.dma_start(kbot[64:128], ktop[0:64])
                  if debug:
                      nc.sync.dma_start(dbg["d_qraw"].ap(), qraw)
                      nc.sync.dma_start(dbg["d_kvraw"].ap(), kvraw)

                  # ---- V_aug: transpose v chunks, append ones column ----
                  nc.vector.memset(vaug[:, :, D:D + 1], 1.0)
                  for c in range(KCH):
                      tp = psacc.tile([128, 512], F32, tag="acc")
                      nc.tensor.transpose(tp[:, 0:64], kvraw[64:128, ts(c, 128)],
                                          idt[64:128, 64:128])
                      nc.vector.tensor_copy(vaug[:, c, 0:D], tp[:, 0:64])

              # ================= phase C: attention =================
              with tc.tile_pool(name="rec", bufs=2) as recp:
                  for j in range(PAIRS):
                      oa = psacc.tile([128, LQ], F32, tag="acc")
                      ob = psacc.tile([128, LQ], F32, tag="acc")
                      for c in range(KCH):
                          st = psst.tile([128, 2, LQ], F32, tag="st")
                          nc.tensor.matmul(st[:, 0, :], ktop[:, ts(c, 128)],
                                           qrot[:, j, :], start=True, stop=True)
                          nc.tensor.matmul(st[:, 1, :], kbot[:, ts(c, 128)],
                                           qrot[:, j, :], start=True, stop=True)
                          pt = ptp.tile([128, 2, LQ], mybir.dt.float16, tag="p")
                          nc.scalar.activation(pt, st,
                                               mybir.ActivationFunctionType.Exp)
                          # mask multiply (all-fp16 -> DVE 2x_1p mode)
                          for tt in range(2):
                              nc.vector.tensor_mul(pt[:, tt, :], pt[:, tt, :],
                                                   mt[:, c, :])
                          if debug and j == 0 and c == 0:
                              nc.sync.dma_start(dbg["d_pt0"].ap(),
                                                pt.bitcast(F32))
                          nc.tensor.matmul(oa[0:D + 1, :], vaug[:, c, :],
                                           pt[:, 0, :], start=(c == 0),
                                           stop=(c == KCH - 1))
                          nc.tensor.matmul(ob[0:D + 1, :], vaug[:, c, :],
                                           pt[:, 1, :], start=(c == 0),
                                           stop=(c == KCH - 1))
                      for tt, op in ((0, oa), (1, ob)):
                          rec = recp.tile([65, LQ], F32R, tag="rec")
                          with nc.allow_low_precision(reason="f32r recip feeds f32r matmul"):
                              nc.vector.reciprocal(rec[64:65], op[D:D + 1, :])
                          rbp = psacc.tile([128, LQ], F32, tag="acc")
                          nc.tensor.matmul(rbp[0:64, :], ones64[64:65, :],
                                           rec[64:65, :], start=True, stop=True)
                          rbs = recp.tile([64, LQ], F32, tag="recb")
                          nc.scalar.copy(rbs, rbp[0:64, :])
                          if tt == 0:
                              nc.vector.tensor_mul(obig[0:64, j, :],
                                                   op[0:D, :], rbs)
                          else:
                              osb = recp.tile([64, LQ], F32R, tag="osb")
                              nc.vector.tensor_mul(osb, op[0:D, :], rbs)
                              nc.gpsimd.dma_start(obig[64:128, j, :], osb)

              # ================= phase D: output projection =================
              with (
                  tc.tile_pool(name="wout", bufs=2) as wout,
                  tc.tile_pool(name="yout", bufs=2) as yout,
              ):
                  for fb in range(FCH):
                      wos = wout.tile([128, FCH, 128], F32R, tag="wo")
                      nc.sync.dma_start(wos, t["wo"].ap()[fb])
                      psy = psacc.tile([128, LQ], F32, tag="acc")
                      for j in range(FCH):
                          nc.tensor.matmul(psy, wos[:, j, :], obig[:, j, :],
                                           start=(j == 0), stop=(j == FCH - 1))
                      ysb = yout.tile([128, LQ], F32, tag="y")
                      nc.vector.tensor_scalar_add(ysb, psy, bo_sb[:, fb:fb + 1])
                      nc.sync.dma_start(yT.ap()[ts(fb, 128), :], ysb)
              if debug:
                  nc.sync.dma_start(dbg["d_qrot"].ap(), qrot.bitcast(F32))
                  nc.sync.dma_start(dbg["d_ktop"].ap(), ktop.bitcast(F32))
                  nc.sync.dma_start(dbg["d_kbot"].ap(), kbot.bitcast(F32))
                  nc.sync.dma_start(dbg["d_vaug"].ap(), vaug.bitcast(F32))
                  nc.sync.dma_start(dbg["d_obig"].ap(), obig.bitcast(F32))

    nc.compile()
    return nc


def _tables():
    """RoPE tables in halves-permuted basis: rows i (even-half) hold +sin,
    rows 32+i (odd-half) hold -sin (for the tmp-then-swap formulation)."""
    inv_freq = 1.0 / (10000.0 ** (np.arange(0, D, 2, dtype=np.float64) / D))
    ang = np.outer(inv_freq, np.arange(L, dtype=np.float64))  # [32, L]
    cos = np.cos(ang).astype(np.float32)
    sin = np.sin(ang).astype(np.float32)
    cos64 = np.concatenate([cos, cos], axis=0)                # [64, L]
    sin_sgn = np.concatenate([sin, -sin], axis=0)             # [64, L]
    return cos64, sin_sgn


def _prep_weights(Wq, bq, Wk, bk, Wv, bv, Wo, bo):
    perm = np.concatenate([np.arange(0, D, 2), np.arange(1, D, 2)])
    WqP = np.asarray(Wq, dtype=np.float32)[:, :, perm].reshape(F, H * D)
    bqP = np.asarray(bq, dtype=np.float32)[:, perm].reshape(H * D)
    WkP = np.asarray(Wk, dtype=np.float32)[:, perm]
    bkP = np.asarray(bk, dtype=np.float32)[perm]
    Wkv = np.concatenate([WkP, np.asarray(Wv, dtype=np.float32)], axis=1)
    bkv = np.concatenate([bkP, np.asarray(bv, dtype=np.float32)])
    WoR = np.asarray(Wo, dtype=np.float32).reshape(H * D, F)
    bo_ = np.asarray(bo, dtype=np.float32)

    wq_pret = round_f32r(np.ascontiguousarray(
        WqP.reshape(FCH, 128, FCH, 128).transpose(2, 1, 0, 3)))
    wkv_pret = round_f32r(np.ascontiguousarray(
        Wkv.reshape(FCH, 128, 128).transpose(1, 0, 2)))
    wo_pret = round_f32r(np.ascontiguousarray(
        WoR.reshape(FCH, 128, FCH, 128).transpose(2, 1, 0, 3)))
    bqbo = np.ascontiguousarray(np.concatenate(
        [bqP.reshape(FCH, 128).T, bo_.reshape(FCH, 128).T], axis=1))
    return wq_pret, wkv_pret, wo_pret, bqbo, bkv


def kernel(inputs_q, inputs_kv, mask, Wq, bq, Wk, bk, Wv, bv, Wo, bo):
    if "nc" not in _CACHED:
        _CACHED["nc"] = build_nc()
    nc = _CACHED["nc"]

    wq_pret, wkv_pret, wo_pret, bqbo, bkv = _prep_weights(
        Wq, bq, Wk, bk, Wv, bv, Wo, bo)

    cos64, sin_sgn = _tables()
    scale = 1.0 / np.sqrt(np.float32(D))
    cksk = np.ascontiguousarray(
        np.concatenate([cos64, sin_sgn], axis=1))      # [64, 2*L] (L=LK)
    cosq_full = np.tile(cos64 * scale, (2, 1))         # [128, L]
    sinq_full = np.tile(sin_sgn * scale, (2, 1))

    xq = np.asarray(inputs_q, dtype=np.float32)
    xkv = np.asarray(inputs_kv, dtype=np.float32)
    mk = np.asarray(mask)

    in_maps = []
    for core in range(NCORES):
        b = core // 4
        qs = (core % 4) * LQ
        xq_t = round_f32r(np.ascontiguousarray(
            xq[b, qs:qs + LQ, :].T.reshape(FCH, 128, LQ)))
        xkv_t = round_f32r(np.ascontiguousarray(
            xkv[b].T.reshape(FCH, 128, NL, LQ).transpose(2, 0, 1, 3)))
        mask_t = np.ascontiguousarray(
            mk[b, 0, qs:qs + LQ, :].T.reshape(KCH, 128, LQ)
            .astype(np.float16))
        in_maps.append({
            "xq_t": xq_t,
            "xkv_t": xkv_t,
            "mask_t": mask_t,
            "wq": wq_pret,
            "wkv": wkv_pret,
            "wo": wo_pret,
            "bqbo": bqbo,
            "bkv": bkv,
            "cosq": np.ascontiguousarray(cosq_full[:, qs:qs + LQ]),
            "sinq": np.ascontiguousarray(sinq_full[:, qs:qs + LQ]),
            "cksk": cksk,
        })

    res = bass_utils.run_bass_kernel_spmd(nc, in_maps,
                                          core_ids=list(range(NCORES)))
    _CACHED["last_results"] = res
    _CACHED["last_maps"] = in_maps

    out = np.empty((B, L, F), dtype=np.float32)
    for core in range(NCORES):
        b = core // 4
        qs = (core % 4) * LQ
        out[b, qs:qs + LQ, :] = res.results[core]["yT"].T
    return out



# revision 6
# speedup vs baseline: 1.0534x; 1.0534x over previous
"""MQA attention (B=2, Lq=Lkv=2048, F=1024, H=16, D=64) on 8 TRN2 cores.

Sharding: core = (batch, query-block-of-512). Each core computes its full
output rows (all 16 heads + output projection) -> no collectives; host
concatenates per-core yT slabs.

v2 dataflow (bf16/fp16 matmul operands, f32 PSUM accumulation):
  KV chain first (per 512-row l-block): kvT = Wkv.T @ xkvT -> RoPE-k
  (halves-permuted basis, swap via small PE matmul) -> ktop/kbot bf16;
  V transposed into vaug fp16 (ones col 64 = softmax denominator row).
  Q proj per head-pair j (interleaved into the attention pair loop):
  qT = Wq_j.T @ xqT -> RoPE -> qrot bf16.
  Attention per (pair j, kv-chunk c): S^T x2 (ktop/kbot stationary) ->
  exp on ACT ([128,2,512] PSUM supertile -> fp16) -> mask mul x2 on DVE
  (fp16 2x mode) -> O accumulation x2 (vaug stationary).
  Pair epilogue off the PE critical path: denominator rows -> DVE
  reciprocal_approx_fast -> fp16 -> K=1 ones matmul broadcast -> DVE
  normalize muls -> obig bf16 (head 2j+1 half moved down via gpsimd DMA).
  yT = Wo-chunks.T @ obig (+bo), Wo fully prefetched during attention.
"""

import ml_dtypes
import numpy as np

import concourse.bass as bass
import concourse.tile as tile
from concourse import bacc, mybir
from concourse import bass_utils
from concourse.bass import ts
from concourse.masks import make_identity

F32 = mybir.dt.float32
BF16 = mybir.dt.bfloat16
FP16 = mybir.dt.float16

B, L, F, H, D = 2, 2048, 1024, 16, 64
LQ = 512            # query rows per core
LK = 2048           # kv rows (full)
NCORES = 8
PAIRS = H // 2      # head pairs (one qT partition block each)
FCH = F // 128      # f contraction chunks
KCH = LK // 128     # lk chunks
NL = LK // LQ       # kv l-blocks

_CACHED = {}


def build_nc():
    nc = bacc.Bacc("TRN2", target_bir_lowering=False, debug=False,
                   num_devices=NCORES)
    dt_in = [
        ("xq_t", [FCH, 128, LQ], BF16),        # [f, p, lq]
        ("xkv_t", [NL, FCH, 128, LQ], BF16),   # [l, f, p, lq]
        ("mask_t", [KCH, 128, LQ], FP16),      # [c, p, lq]
        ("wq", [FCH, 128, FCH, 128], BF16),    # [j, p, f, m]
        ("wkv", [128, FCH, 128], BF16),        # [p, f, m]
        ("wo", [FCH, 128, FCH, 128], BF16),    # [fb, p, j, m]
        ("bqbo", [128, 2 * FCH], F32),         # cols 0:8 bq-blocks, 8:16 bo
        ("bkv", [2 * D], F32),
        ("cosq", [128, LQ], F32),
        ("sinq", [128, LQ], F32),
        ("cksk", [D, 2 * LK], F32),            # [p, (cos|sin)*lk]
    ]
    t = {name: nc.dram_tensor(name, shape, dt, kind="ExternalInput")
         for name, shape, dt in dt_in}
    yT = nc.dram_tensor("yT", [F, LQ], F32, kind="ExternalOutput")

    with tile.TileContext(nc) as tc:
        with (
            tc.tile_pool(name="persist", bufs=1) as persist,
            tc.tile_pool(name="ptiles", bufs=3) as ptp,
            tc.tile_pool(name="small", bufs=4) as small,
            tc.tile_pool(name="xin", bufs=2) as xin,
            tc.tile_pool(name="wst", bufs=2) as wst,
            tc.tile_pool(name="qraw", bufs=2) as qrp,
            tc.tile_pool(name="kvraw", bufs=2) as kvp,
            tc.tile_pool(name="ropetmp", bufs=2) as rtp,
            tc.tile_pool(name="rec", bufs=2) as recp,
            tc.tile_pool(name="yout", bufs=2) as yout,
            tc.tile_pool(name="psa", bufs=2, space="PSUM") as psa,   # 2 banks
            tc.tile_pool(name="psb", bufs=2, space="PSUM") as psb,   # 2 banks
            tc.tile_pool(name="psst", bufs=2, space="PSUM") as psst,  # 4 banks
        ):
            # ---------------- small constants (gpsimd DMA queue) ---------
            cq = persist.tile([128, LQ], F32)
            sq = persist.tile([128, LQ], F32)
            cksk = persist.tile([D, 2, LK], F32)
            nc.gpsimd.dma_start(cq, t["cosq"].ap())
            nc.gpsimd.dma_start(sq, t["sinq"].ap())
            nc.gpsimd.dma_start(cksk,
                                t["cksk"].ap().rearrange("p (a l) -> p a l", a=2))
            ck = cksk[:, 0, :]
            sk = cksk[:, 1, :]
            bqbo = small.tile([128, 2 * FCH], F32, tag="bias")
            nc.gpsimd.dma_start(bqbo, t["bqbo"].ap())
            bq_sb = bqbo[:, 0:FCH]
            bo_sb = bqbo[:, FCH:2 * FCH]
            bkv_sb = small.tile([128, 1], F32, tag="bias2")
            nc.gpsimd.dma_start(bkv_sb, t["bkv"].ap().unsqueeze(1))

            # mask chunks (needed from attention start; ~2MB)
            mt = persist.tile([128, KCH, LQ], FP16)
            for c in range(KCH):
                nc.gpsimd.dma_start(mt[:, c, :], t["mask_t"].ap()[c])

            # Wo prefetch (needed only in phase D; ~2MB)
            wo_sb = persist.tile([128, FCH, FCH, 128], BF16)
            for fb in range(FCH):
                nc.gpsimd.dma_start(wo_sb[:, fb], t["wo"].ap()[fb])

            # ---------------- persistent compute tiles -------------------
            qrot = persist.tile([128, PAIRS, LQ], BF16)
            ktop = persist.tile([128, LK], BF16)          # k rows 0:64
            kbot = persist.tile([128, LK], BF16)          # k rows 64:128
            vaug = persist.tile([128, KCH, D + 1], FP16)  # V chunks + ones col
            obig = persist.tile([128, PAIRS, LQ], BF16)   # normalized O^T

            idt = small.tile([128, 128], F32, tag="ident")
            make_identity(nc, idt)
            # halves-swap permutation matrix: M[p, p-xor-32-within-head] = 1
            swpf = small.tile([128, 128], F32, tag="swpf")
            nc.gpsimd.memset(swpf, 0.0)
            for o1, o2 in ((0, 32), (32, 0), (64, 96), (96, 64)):
                nc.gpsimd.affine_select(
                    out=swpf[o1:o1 + 32, o2:o2 + 32],
                    in_=swpf[o1:o1 + 32, o2:o2 + 32],
                    compare_op=mybir.AluOpType.not_equal, fill=1.0,
                    base=0, pattern=[[-1, 32]], channel_multiplier=1)
            swp = small.tile([128, 128], BF16, tag="swp")
            nc.vector.tensor_copy(swp, swpf)
            ones2 = small.tile([1, D], FP16, tag="ones2")
            nc.vector.memset(ones2, 1.0)

            nc.vector.memset(ktop[64:128], 0.0)
            nc.vector.memset(kbot[0:64], 0.0)
            nc.vector.memset(vaug[:, :, D:D + 1], 1.0)

            # ================= phase A: KV chain per l-block ==============
            wkv_sb = wst.tile([128, FCH, 128], BF16, tag="wkv")
            nc.sync.dma_start(wkv_sb, t["wkv"].ap())
            for l in range(NL):
                xkv = xin.tile([128, FCH, LQ], BF16, tag="x")
                for f in range(FCH):
                    nc.sync.dma_start(xkv[:, f, :], t["xkv_t"].ap()[l, f])
                pkv = psb.tile([128, LQ], F32, tag="b")
                for f in range(FCH):
                    nc.tensor.matmul(pkv, wkv_sb[:, f, :], xkv[:, f, :],
                                     start=(f == 0), stop=(f == FCH - 1))
                kvl = kvp.tile([128, LQ], F32, tag="kv")
                nc.vector.tensor_scalar_add(kvl, pkv, bkv_sb[:, 0:1])

                # RoPE on k rows 0:64: krot = k*cos + Swap @ (k*sin_signed)
                lsl = ts(l, LQ)
                tmk = rtp.tile([D, LQ], BF16, tag="ksin")
                nc.vector.tensor_mul(tmk, kvl[0:64], sk[:, lsl])
                kc = rtp.tile([D, LQ], F32, tag="kcos")
                nc.vector.tensor_mul(kc, kvl[0:64], ck[:, lsl])
                pswk = psa.tile([128, LQ], F32, tag="a")
                nc.tensor.matmul(pswk[0:64], swp[0:64, 0:64], tmk,
                                 start=True, stop=True)
                nc.vector.tensor_add(ktop[0:64, lsl], kc, pswk[0:64])
                nc.gpsimd.dma_start(kbot[64:128, lsl], ktop[0:64, lsl])

                # V transpose into vaug chunks (+ copy on idle ACT engine)
                for ci in range(4):
                    c = 4 * l + ci
                    tp = psa.tile([128, LQ], F32, tag="a")
                    nc.tensor.transpose(tp[:, 0:64], kvl[64:128, ts(ci, 128)],
                                        idt[64:128, 64:128])
                    nc.scalar.copy(vaug[:, c, 0:D], tp[:, 0:64])

            # ---- Q projection + RoPE for one head-pair ----
            xq = persist.tile([128, FCH, LQ], BF16)
            for f in range(FCH):
                nc.sync.dma_start(xq[:, f, :], t["xq_t"].ap()[f])

            def q_proj(j):
                wq_j = wst.tile([128, FCH, 128], BF16, tag="wq")
                nc.sync.dma_start(wq_j, t["wq"].ap()[j])
                psq = psa.tile([128, LQ], F32, tag="a")
                for f in range(FCH):
                    nc.tensor.matmul(psq, wq_j[:, f, :], xq[:, f, :],
                                     start=(f == 0), stop=(f == FCH - 1))
                qraw = qrp.tile([128, LQ], F32, tag="q")
                nc.vector.tensor_scalar_add(qraw, psq, bq_sb[:, j:j + 1])
                tmq = rtp.tile([128, LQ], BF16, tag="qsin")
                nc.vector.tensor_mul(tmq, qraw, sq)
                psw = psa.tile([128, LQ], F32, tag="a")
                nc.tensor.matmul(psw, swp, tmq, start=True, stop=True)
                qc = rtp.tile([128, LQ], F32, tag="qcos")
                nc.vector.tensor_mul(qc, qraw, cq)
                nc.vector.tensor_add(qrot[:, j, :], qc, psw)

            q_proj(0)

            # ================= phase C: attention =================
            for j in range(PAIRS):
                oa = psa.tile([128, LQ], F32, tag="a")
                ob = psb.tile([128, LQ], F32, tag="b")
                for c in range(KCH):
                    st = psst.tile([128, 2, LQ], F32, tag="st")
                    nc.tensor.matmul(st[:, 0, :], ktop[:, ts(c, 128)],
                                     qrot[:, j, :], start=True, stop=True)
                    nc.tensor.matmul(st[:, 1, :], kbot[:, ts(c, 128)],
                                     qrot[:, j, :], start=True, stop=True)
                    pt = ptp.tile([128, 2, LQ], FP16, tag="p")
                    nc.scalar.activation(pt, st,
                                         mybir.ActivationFunctionType.Exp)
                    for tt in range(2):
                        nc.vector.tensor_mul(pt[:, tt, :], pt[:, tt, :],
                                             mt[:, c, :])
                    nc.tensor.matmul(oa[0:D + 1, :], vaug[:, c, :],
                                     pt[:, 0, :], start=(c == 0),
                                     stop=(c == KCH - 1))
                    nc.tensor.matmul(ob[0:D + 1, :], vaug[:, c, :],
                                     pt[:, 1, :], start=(c == 0),
                                     stop=(c == KCH - 1))
                    # interleave next pair's Q projection mid-pair
                    if c == 7 and j + 1 < PAIRS:
                        q_proj(j + 1)

                # ---- pair epilogue: batched reciprocal normalize ----
                den = recp.tile([1, 2, LQ], F32, tag="den")
                nc.vector.tensor_copy(den[:, 0, :], oa[D:D + 1, :])
                nc.vector.tensor_copy(den[:, 1, :], ob[D:D + 1, :])
                rcf = recp.tile([1, 2, LQ], F32, tag="rcf")
                nc.vector.reciprocal_approx_fast(rcf, den)
                rch = recp.tile([1, 2, LQ], FP16, tag="rch")
                nc.gpsimd.tensor_copy(rch, rcf)
                rbp = psst.tile([128, 2, LQ], F32, tag="st")
                for tt in range(2):
                    nc.tensor.matmul(rbp[0:D, tt, :], ones2, rch[0:1, tt, :],
                                     start=True, stop=True)
                rbs = recp.tile([D, 2, LQ], FP16, tag="rbs")
                nc.vector.tensor_copy(rbs, rbp[0:D, :, :])
                nc.vector.tensor_mul(obig[0:D, j, :], oa[0:D, :],
                                     rbs[:, 0, :])
                osb = recp.tile([D, LQ], BF16, tag="osb")
                nc.vector.tensor_mul(osb, ob[0:D, :], rbs[:, 1, :])
                nc.gpsimd.dma_start(obig[64:128, j, :], osb)

            # ================= phase D: output projection =================
            for fb in range(FCH):
                psy = psa.tile([128, LQ], F32, tag="a")
                for j in range(FCH):
                    nc.tensor.matmul(psy, wo_sb[:, fb, j, :], obig[:, j, :],
                                     start=(j == 0), stop=(j == FCH - 1))
                ysb = yout.tile([128, LQ], F32, tag="y")
                nc.vector.tensor_scalar_add(ysb, psy, bo_sb[:, fb:fb + 1])
                nc.sync.dma_start(yT.ap()[ts(fb, 128), :], ysb)

    nc.compile()
    return nc


def _tables():
    """RoPE tables in halves-permuted basis: rows i (even-half) hold +sin,
    rows 32+i (odd-half) hold -sin (for the tmp-then-swap formulation)."""
    inv_freq = 1.0 / (10000.0 ** (np.arange(0, D, 2, dtype=np.float64) / D))
    ang = np.outer(inv_freq, np.arange(L, dtype=np.float64))  # [32, L]
    cos = np.cos(ang).astype(np.float32)
    sin = np.sin(ang).astype(np.float32)
    cos64 = np.concatenate([cos, cos], axis=0)                # [64, L]
    sin_sgn = np.concatenate([sin, -sin], axis=0)             # [64, L]
    return cos64, sin_sgn


def _prep_weights(Wq, bq, Wk, bk, Wv, bv, Wo, bo):
    perm = np.concatenate([np.arange(0, D, 2), np.arange(1, D, 2)])
    WqP = np.asarray(Wq, dtype=np.float32)[:, :, perm].reshape(F, H * D)
    bqP = np.asarray(bq, dtype=np.float32)[:, perm].reshape(H * D)
    WkP = np.asarray(Wk, dtype=np.float32)[:, perm]
    bkP = np.asarray(bk, dtype=np.float32)[perm]
    Wkv = np.concatenate([WkP, np.asarray(Wv, dtype=np.float32)], axis=1)
    bkv = np.concatenate([bkP, np.asarray(bv, dtype=np.float32)])
    WoR = np.asarray(Wo, dtype=np.float32).reshape(H * D, F)
    bo_ = np.asarray(bo, dtype=np.float32)

    wq_pret = np.ascontiguousarray(
        WqP.reshape(FCH, 128, FCH, 128).transpose(2, 1, 0, 3)).astype(
            ml_dtypes.bfloat16)
    wkv_pret = np.ascontiguousarray(
        Wkv.reshape(FCH, 128, 128).transpose(1, 0, 2)).astype(
            ml_dtypes.bfloat16)
    wo_pret = np.ascontiguousarray(
        WoR.reshape(FCH, 128, FCH, 128).transpose(2, 1, 0, 3)).astype(
            ml_dtypes.bfloat16)
    bqbo = np.ascontiguousarray(np.concatenate(
        [bqP.reshape(FCH, 128).T, bo_.reshape(FCH, 128).T], axis=1))
    return wq_pret, wkv_pret, wo_pret, bqbo, bkv


def kernel(inputs_q, inputs_kv, mask, Wq, bq, Wk, bk, Wv, bv, Wo, bo):
    if "nc" not in _CACHED:
        _CACHED["nc"] = build_nc()
    nc = _CACHED["nc"]

    wq_pret, wkv_pret, wo_pret, bqbo, bkv = _prep_weights(
        Wq, bq, Wk, bk, Wv, bv, Wo, bo)

    cos64, sin_sgn = _tables()
    scale = 1.0 / np.sqrt(np.float32(D))
    cksk = np.ascontiguousarray(
        np.concatenate([cos64, sin_sgn], axis=1))      # [64, 2*L] (L=LK)
    cosq_full = np.tile(cos64 * scale, (2, 1))         # [128, L]
    sinq_full = np.tile(sin_sgn * scale, (2, 1))

    xq = np.asarray(inputs_q, dtype=np.float32)
    xkv = np.asarray(inputs_kv, dtype=np.float32)
    mk = np.asarray(mask)

    in_maps = []
    for core in range(NCORES):
        b = core // 4
        qs = (core % 4) * LQ
        xq_t = np.ascontiguousarray(
            xq[b, qs:qs + LQ, :].T.reshape(FCH, 128, LQ)).astype(
                ml_dtypes.bfloat16)
        xkv_t = np.ascontiguousarray(
            xkv[b].T.reshape(FCH, 128, NL, LQ).transpose(2, 0, 1, 3)).astype(
                ml_dtypes.bfloat16)
        mask_t = np.ascontiguousarray(
            mk[b, 0, qs:qs + LQ, :].T.reshape(KCH, 128, LQ)
            .astype(np.float16))
        in_maps.append({
            "xq_t": xq_t,
            "xkv_t": xkv_t,
            "mask_t": mask_t,
            "wq": wq_pret,
            "wkv": wkv_pret,
            "wo": wo_pret,
            "bqbo": bqbo,
            "bkv": bkv,
            "cosq": np.ascontiguousarray(cosq_full[:, qs:qs + LQ]),
            "sinq": np.ascontiguousarray(sinq_full[:, qs:qs + LQ]),
            "cksk": cksk,
        })

    res = bass_utils.run_bass_kernel_spmd(nc, in_maps,
                                          core_ids=list(range(NCORES)))
    _CACHED["last_results"] = res
    _CACHED["last_maps"] = in_maps

    out = np.empty((B, L, F), dtype=np.float32)
    for core in range(NCORES):
        b = core // 4
        qs = (core % 4) * LQ
        out[b, qs:qs + LQ, :] = res.results[core]["yT"].T
    return out


# revision 7
# speedup vs baseline: 1.0645x; 1.0105x over previous
"""MQA attention (B=2, Lq=Lkv=2048, F=1024, H=16, D=64) on 8 TRN2 cores.

Sharding: core = (batch, query-block-of-512). Each core computes its full
output rows (all 16 heads + output projection) -> no collectives; host
concatenates per-core yT slabs.

v2 dataflow (bf16/fp16 matmul operands, f32 PSUM accumulation):
  KV chain first (per 512-row l-block): kvT = Wkv.T @ xkvT -> RoPE-k
  (halves-permuted basis, swap via small PE matmul) -> ktop/kbot bf16;
  V transposed into vaug fp16 (ones col 64 = softmax denominator row).
  Q proj per head-pair j (interleaved into the attention pair loop):
  qT = Wq_j.T @ xqT -> RoPE -> qrot bf16.
  Attention per (pair j, kv-chunk c): S^T x2 (ktop/kbot stationary) ->
  exp on ACT ([128,2,512] PSUM supertile -> fp16) -> mask mul x2 on DVE
  (fp16 2x mode) -> O accumulation x2 (vaug stationary).
  Pair epilogue off the PE critical path: denominator rows -> DVE
  reciprocal_approx_fast -> fp16 -> K=1 ones matmul broadcast -> DVE
  normalize muls -> obig bf16 (head 2j+1 half moved down via gpsimd DMA).
  yT = Wo-chunks.T @ obig (+bo), Wo fully prefetched during attention.
"""

import ml_dtypes
import numpy as np

import concourse.bass as bass
import concourse.tile as tile
from concourse import bacc, mybir
from concourse import bass_utils
from concourse.bass import ts
from concourse.masks import make_identity

F32 = mybir.dt.float32
BF16 = mybir.dt.bfloat16
FP16 = mybir.dt.float16

B, L, F, H, D = 2, 2048, 1024, 16, 64
LQ = 512            # query rows per core
LK = 2048           # kv rows (full)
NCORES = 8
PAIRS = H // 2      # head pairs (one qT partition block each)
FCH = F // 128      # f contraction chunks
KCH = LK // 128     # lk chunks
NL = LK // LQ       # kv l-blocks

_CACHED = {}


def build_nc():
    nc = bacc.Bacc("TRN2", target_bir_lowering=False, debug=False,
                   num_devices=NCORES)
    dt_in = [
        ("xq_t", [FCH, 128, LQ], BF16),        # [f, p, lq]
        ("xkv_t", [NL, FCH, 128, LQ], BF16),   # [l, f, p, lq]
        ("mask_t", [KCH, 128, LQ], FP16),      # [c, p, lq]
        ("wq", [FCH, 128, FCH, 128], BF16),    # [j, p, f, m]
        ("wkv", [128, FCH, 128], BF16),        # [p, f, m]
        ("wo", [FCH, 128, FCH, 128], BF16),    # [fb, p, j, m]
        ("bqbo", [128, 2 * FCH], F32),         # cols 0:8 bq-blocks, 8:16 bo
        ("bkv", [2 * D], F32),
        ("cosq", [128, LQ], F32),
        ("sinq", [128, LQ], F32),
        ("cksk", [D, 2 * LK], F32),            # [p, (cos|sin)*lk]
    ]
    t = {name: nc.dram_tensor(name, shape, dt, kind="ExternalInput")
         for name, shape, dt in dt_in}
    yT = nc.dram_tensor("yT", [F, LQ], F32, kind="ExternalOutput")

    with tile.TileContext(nc) as tc:
        with (
            tc.tile_pool(name="persist", bufs=1) as persist,
            tc.tile_pool(name="ptiles", bufs=3) as ptp,
            tc.tile_pool(name="small", bufs=4) as small,
            tc.tile_pool(name="xin", bufs=2) as xin,
            tc.tile_pool(name="wst", bufs=2) as wst,
            tc.tile_pool(name="qraw", bufs=2) as qrp,
            tc.tile_pool(name="kvraw", bufs=2) as kvp,
            tc.tile_pool(name="ropetmp", bufs=2) as rtp,
            tc.tile_pool(name="rec", bufs=2) as recp,
            tc.tile_pool(name="yout", bufs=2) as yout,
            tc.tile_pool(name="psa", bufs=2, space="PSUM") as psa,   # 2 banks
            tc.tile_pool(name="psb", bufs=2, space="PSUM") as psb,   # 2 banks
            tc.tile_pool(name="psst", bufs=2, space="PSUM") as psst,  # 4 banks
        ):
            # ---------------- small constants (gpsimd DMA queue) ---------
            cq = persist.tile([128, LQ], F32)
            sq = persist.tile([128, LQ], F32)
            cksk = persist.tile([D, 2, LK], F32)
            nc.gpsimd.dma_start(cq, t["cosq"].ap())
            nc.gpsimd.dma_start(sq, t["sinq"].ap())
            nc.gpsimd.dma_start(cksk,
                                t["cksk"].ap().rearrange("p (a l) -> p a l", a=2))
            ck = cksk[:, 0, :]
            sk = cksk[:, 1, :]
            bqbo = small.tile([128, 2 * FCH], F32, tag="bias")
            nc.gpsimd.dma_start(bqbo, t["bqbo"].ap())
            bq_sb = bqbo[:, 0:FCH]
            bo_sb = bqbo[:, FCH:2 * FCH]
            bkv_sb = small.tile([128, 1], F32, tag="bias2")
            nc.gpsimd.dma_start(bkv_sb, t["bkv"].ap().unsqueeze(1))

            # mask chunks (needed from attention start; ~2MB)
            mt = persist.tile([128, KCH, LQ], FP16)
            for c in range(KCH):
                nc.gpsimd.dma_start(mt[:, c, :], t["mask_t"].ap()[c])

            # Wo prefetch (needed only in phase D; ~2MB)
            wo_sb = persist.tile([128, FCH, FCH, 128], BF16)
            for fb in range(FCH):
                nc.gpsimd.dma_start(wo_sb[:, fb], t["wo"].ap()[fb])

            # ---------------- persistent compute tiles -------------------
            qrot = persist.tile([128, PAIRS, LQ], BF16)
            ktop = persist.tile([128, LK], BF16)          # k rows 0:64
            kbot = persist.tile([128, LK], BF16)          # k rows 64:128
            vaug = persist.tile([128, KCH, D + 1], FP16)  # V chunks + ones col
            obig = persist.tile([128, PAIRS, LQ], BF16)   # normalized O^T

            idt = small.tile([128, 128], F32, tag="ident")
            make_identity(nc, idt)
            # halves-swap permutation matrix: M[p, p-xor-32-within-head] = 1
            swpf = small.tile([128, 128], F32, tag="swpf")
            nc.gpsimd.memset(swpf, 0.0)
            for o1, o2 in ((0, 32), (32, 0), (64, 96), (96, 64)):
                nc.gpsimd.affine_select(
                    out=swpf[o1:o1 + 32, o2:o2 + 32],
                    in_=swpf[o1:o1 + 32, o2:o2 + 32],
                    compare_op=mybir.AluOpType.not_equal, fill=1.0,
                    base=0, pattern=[[-1, 32]], channel_multiplier=1)
            swp = small.tile([128, 128], BF16, tag="swp")
            nc.vector.tensor_copy(swp, swpf)
            ones2 = small.tile([1, D], FP16, tag="ones2")
            nc.vector.memset(ones2, 1.0)

            nc.vector.memset(ktop[64:128], 0.0)
            nc.vector.memset(kbot[0:64], 0.0)
            nc.vector.memset(vaug[:, :, D:D + 1], 1.0)

            # ================= phase A: KV chain per l-block ==============
            wkv_sb = wst.tile([128, FCH, 128], BF16, tag="wkv")
            nc.sync.dma_start(wkv_sb, t["wkv"].ap())
            for l in range(NL):
                xkv = xin.tile([128, FCH, LQ], BF16, tag="x")
                for f in range(FCH):
                    nc.sync.dma_start(xkv[:, f, :], t["xkv_t"].ap()[l, f])
                pkv = psb.tile([128, LQ], F32, tag="b")
                for f in range(FCH):
                    nc.tensor.matmul(pkv, wkv_sb[:, f, :], xkv[:, f, :],
                                     start=(f == 0), stop=(f == FCH - 1))
                kvl = kvp.tile([128, LQ], F32, tag="kv")
                nc.vector.tensor_scalar_add(kvl, pkv, bkv_sb[:, 0:1])

                # RoPE on k rows 0:64: krot = k*cos + Swap @ (k*sin_signed)
                lsl = ts(l, LQ)
                tmk = rtp.tile([D, LQ], BF16, tag="ksin")
                nc.vector.tensor_mul(tmk, kvl[0:64], sk[:, lsl])
                kc = rtp.tile([D, LQ], F32, tag="kcos")
                nc.vector.tensor_mul(kc, kvl[0:64], ck[:, lsl])
                pswk = psa.tile([128, LQ], F32, tag="a")
                nc.tensor.matmul(pswk[0:64], swp[0:64, 0:64], tmk,
                                 start=True, stop=True)
                nc.vector.tensor_add(ktop[0:64, lsl], kc, pswk[0:64])
                nc.gpsimd.dma_start(kbot[64:128, lsl], ktop[0:64, lsl])

                # V transpose into vaug chunks (+ copy on idle ACT engine)
                for ci in range(4):
                    c = 4 * l + ci
                    tp = psa.tile([128, LQ], F32, tag="a")
                    nc.tensor.transpose(tp[:, 0:64], kvl[64:128, ts(ci, 128)],
                                        idt[64:128, 64:128])
                    nc.scalar.copy(vaug[:, c, 0:D], tp[:, 0:64])

            # ---- Q projection + RoPE for one head-pair ----
            xq = persist.tile([128, FCH, LQ], BF16)
            for f in range(FCH):
                nc.sync.dma_start(xq[:, f, :], t["xq_t"].ap()[f])

            def q_proj(j):
                wq_j = wst.tile([128, FCH, 128], BF16, tag="wq")
                nc.sync.dma_start(wq_j, t["wq"].ap()[j])
                psq = psa.tile([128, LQ], F32, tag="a")
                for f in range(FCH):
                    nc.tensor.matmul(psq, wq_j[:, f, :], xq[:, f, :],
                                     start=(f == 0), stop=(f == FCH - 1))
                qraw = qrp.tile([128, LQ], F32, tag="q")
                nc.vector.tensor_scalar_add(qraw, psq, bq_sb[:, j:j + 1])
                tmq = rtp.tile([128, LQ], BF16, tag="qsin")
                nc.vector.tensor_mul(tmq, qraw, sq)
                psw = psa.tile([128, LQ], F32, tag="a")
                nc.tensor.matmul(psw, swp, tmq, start=True, stop=True)
                qc = rtp.tile([128, LQ], F32, tag="qcos")
                nc.vector.tensor_mul(qc, qraw, cq)
                nc.vector.tensor_add(qrot[:, j, :], qc, psw)

            q_proj(0)

            # ================= phase C: attention =================
            for j in range(PAIRS):
                oa = psa.tile([128, LQ], F32, tag="a")
                ob = psb.tile([128, LQ], F32, tag="b")
                prev_pt = None

                def flush_o(c, pt):
                    nc.tensor.matmul(oa[0:D + 1, :], vaug[:, c, :],
                                     pt[:, 0, :], start=(c == 0),
                                     stop=(c == KCH - 1))
                    nc.tensor.matmul(ob[0:D + 1, :], vaug[:, c, :],
                                     pt[:, 1, :], start=(c == 0),
                                     stop=(c == KCH - 1))

                for c in range(KCH):
                    st = psst.tile([128, 2, LQ], F32, tag="st")
                    nc.tensor.matmul(st[:, 0, :], ktop[:, ts(c, 128)],
                                     qrot[:, j, :], start=True, stop=True)
                    nc.tensor.matmul(st[:, 1, :], kbot[:, ts(c, 128)],
                                     qrot[:, j, :], start=True, stop=True)
                    pt = ptp.tile([128, 2, LQ], FP16, tag="p")
                    nc.scalar.activation(pt, st,
                                         mybir.ActivationFunctionType.Exp)
                    for tt in range(2):
                        nc.vector.tensor_mul(pt[:, tt, :], pt[:, tt, :],
                                             mt[:, c, :])
                    # software pipeline: O for chunk c-1 goes behind the
                    # S matmuls of chunk c so the in-order PE queue never
                    # blocks on the exp->mask round-trip of the same chunk
                    if prev_pt is not None:
                        flush_o(c - 1, prev_pt)
                    prev_pt = pt
                    # interleave next pair's Q projection mid-pair
                    if c == 7 and j + 1 < PAIRS:
                        q_proj(j + 1)
                flush_o(KCH - 1, prev_pt)

                # ---- pair epilogue: batched reciprocal normalize ----
                den = recp.tile([1, 2, LQ], F32, tag="den")
                nc.vector.tensor_copy(den[:, 0, :], oa[D:D + 1, :])
                nc.vector.tensor_copy(den[:, 1, :], ob[D:D + 1, :])
                rcf = recp.tile([1, 2, LQ], F32, tag="rcf")
                nc.vector.reciprocal_approx_fast(rcf, den)
                rch = recp.tile([1, 2, LQ], FP16, tag="rch")
                nc.gpsimd.tensor_copy(rch, rcf)
                rbp = psst.tile([128, 2, LQ], F32, tag="st")
                for tt in range(2):
                    nc.tensor.matmul(rbp[0:D, tt, :], ones2, rch[0:1, tt, :],
                                     start=True, stop=True)
                rbs = recp.tile([D, 2, LQ], FP16, tag="rbs")
                nc.vector.tensor_copy(rbs, rbp[0:D, :, :])
                nc.vector.tensor_mul(obig[0:D, j, :], oa[0:D, :],
                                     rbs[:, 0, :])
                osb = recp.tile([D, LQ], BF16, tag="osb")
                nc.vector.tensor_mul(osb, ob[0:D, :], rbs[:, 1, :])
                nc.gpsimd.dma_start(obig[64:128, j, :], osb)

            # ================= phase D: output projection =================
            for fb in range(FCH):
                psy = psa.tile([128, LQ], F32, tag="a")
                for j in range(FCH):
                    nc.tensor.matmul(psy, wo_sb[:, fb, j, :], obig[:, j, :],
                                     start=(j == 0), stop=(j == FCH - 1))
                ysb = yout.tile([128, LQ], F32, tag="y")
                nc.vector.tensor_scalar_add(ysb, psy, bo_sb[:, fb:fb + 1])
                nc.sync.dma_start(yT.ap()[ts(fb, 128), :], ysb)

    nc.compile()
    return nc


def _tables():
    """RoPE tables in halves-permuted basis: rows i (even-half) hold +sin,
    rows 32+i (odd-half) hold -sin (for the tmp-then-swap formulation)."""
    inv_freq = 1.0 / (10000.0 ** (np.arange(0, D, 2, dtype=np.float64) / D))
    ang = np.outer(inv_freq, np.arange(L, dtype=np.float64))  # [32, L]
    cos = np.cos(ang).astype(np.float32)
    sin = np.sin(ang).astype(np.float32)
    cos64 = np.concatenate([cos, cos], axis=0)                # [64, L]
    sin_sgn = np.concatenate([sin, -sin], axis=0)             # [64, L]
    return cos64, sin_sgn


def _prep_weights(Wq, bq, Wk, bk, Wv, bv, Wo, bo):
    perm = np.concatenate([np.arange(0, D, 2), np.arange(1, D, 2)])
    WqP = np.asarray(Wq, dtype=np.float32)[:, :, perm].reshape(F, H * D)
    bqP = np.asarray(bq, dtype=np.float32)[:, perm].reshape(H * D)
    WkP = np.asarray(Wk, dtype=np.float32)[:, perm]
    bkP = np.asarray(bk, dtype=np.float32)[perm]
    Wkv = np.concatenate([WkP, np.asarray(Wv, dtype=np.float32)], axis=1)
    bkv = np.concatenate([bkP, np.asarray(bv, dtype=np.float32)])
    WoR = np.asarray(Wo, dtype=np.float32).reshape(H * D, F)
    bo_ = np.asarray(bo, dtype=np.float32)

    wq_pret = np.ascontiguousarray(
        WqP.reshape(FCH, 128, FCH, 128).transpose(2, 1, 0, 3)).astype(
            ml_dtypes.bfloat16)
    wkv_pret = np.ascontiguousarray(
        Wkv.reshape(FCH, 128, 128).transpose(1, 0, 2)).astype(
            ml_dtypes.bfloat16)
    wo_pret = np.ascontiguousarray(
        WoR.reshape(FCH, 128, FCH, 128).transpose(2, 1, 0, 3)).astype(
            ml_dtypes.bfloat16)
    bqbo = np.ascontiguousarray(np.concatenate(
        [bqP.reshape(FCH, 128).T, bo_.reshape(FCH, 128).T], axis=1))
    return wq_pret, wkv_pret, wo_pret, bqbo, bkv


def kernel(inputs_q, inputs_kv, mask, Wq, bq, Wk, bk, Wv, bv, Wo, bo):
    if "nc" not in _CACHED:
        _CACHED["nc"] = build_nc()
    nc = _CACHED["nc"]

    wq_pret, wkv_pret, wo_pret, bqbo, bkv = _prep_weights(
        Wq, bq, Wk, bk, Wv, bv, Wo, bo)

    cos64, sin_sgn = _tables()
    scale = 1.0 / np.sqrt(np.float32(D))
    cksk = np.ascontiguousarray(
        np.concatenate([cos64, sin_sgn], axis=1))      # [64, 2*L] (L=LK)
    cosq_full = np.tile(cos64 * scale, (2, 1))         # [128, L]
    sinq_full = np.tile(sin_sgn * scale, (2, 1))

    xq = np.asarray(inputs_q, dtype=np.float32)
    xkv = np.asarray(inputs_kv, dtype=np.float32)
    mk = np.asarray(mask)

    in_maps = []
    for core in range(NCORES):
        b = core // 4
        qs = (core % 4) * LQ
        xq_t = np.ascontiguousarray(
            xq[b, qs:qs + LQ, :].T.reshape(FCH, 128, LQ)).astype(
                ml_dtypes.bfloat16)
        xkv_t = np.ascontiguousarray(
            xkv[b].T.reshape(FCH, 128, NL, LQ).transpose(2, 0, 1, 3)).astype(
                ml_dtypes.bfloat16)
        mask_t = np.ascontiguousarray(
            mk[b, 0, qs:qs + LQ, :].T.reshape(KCH, 128, LQ)
            .astype(np.float16))
        in_maps.append({
            "xq_t": xq_t,
            "xkv_t": xkv_t,
            "mask_t": mask_t,
            "wq": wq_pret,
            "wkv": wkv_pret,
            "wo": wo_pret,
            "bqbo": bqbo,
            "bkv": bkv,
            "cosq": np.ascontiguousarray(cosq_full[:, qs:qs + LQ]),
            "sinq": np.ascontiguousarray(sinq_full[:, qs:qs + LQ]),
            "cksk": cksk,
        })

    res = bass_utils.run_bass_kernel_spmd(nc, in_maps,
                                          core_ids=list(range(NCORES)))
    _CACHED["last_results"] = res
    _CACHED["last_maps"] = in_maps

    out = np.empty((B, L, F), dtype=np.float32)
    for core in range(NCORES):
        b = core // 4
        qs = (core % 4) * LQ
        out[b, qs:qs + LQ, :] = res.results[core]["yT"].T
    return out


# revision 22
# speedup vs baseline: 1.1390x; 1.0700x over previous
"""MQA attention (B=2, Lq=Lkv=2048, F=1024, H=16, D=64) on 8 TRN2 cores.

Sharding: core = (batch, query-block-of-512). Each core computes its full
output rows (all 16 heads + output projection) -> no collectives; host
concatenates per-core yT slabs.

v2 dataflow (bf16/fp16 matmul operands, f32 PSUM accumulation):
  KV chain first (per 512-row l-block): kvT = Wkv.T @ xkvT -> RoPE-k
  (halves-permuted basis, swap via small PE matmul) -> ktop/kbot bf16;
  V transposed into vaug fp16 (ones col 64 = softmax denominator row).
  Q proj per head-pair j (interleaved into the attention pair loop):
  qT = Wq_j.T @ xqT -> RoPE -> qrot bf16.
  Attention per (pair j, kv-chunk c): S^T x2 (ktop/kbot stationary) ->
  exp on ACT ([128,2,512] PSUM supertile -> fp16) -> mask mul x2 on DVE
  (fp16 2x mode) -> O accumulation x2 (vaug stationary).
  Pair epilogue off the PE critical path: denominator rows -> DVE
  reciprocal_approx_fast -> fp16 -> K=1 ones matmul broadcast -> DVE
  normalize muls -> obig bf16 (head 2j+1 half moved down via gpsimd DMA).
  yT = Wo-chunks.T @ obig (+bo), Wo fully prefetched during attention.
"""

import ml_dtypes
import numpy as np

import concourse.bass as bass
import concourse.tile as tile
from concourse import bacc, mybir
from concourse import bass_utils
from concourse.bass import ts
from concourse.masks import make_identity

F32 = mybir.dt.float32
BF16 = mybir.dt.bfloat16
FP16 = mybir.dt.float16

B, L, F, H, D = 2, 2048, 1024, 16, 64
LQ = 512            # query rows per core
LK = 2048           # kv rows (full)
NCORES = 8
PAIRS = H // 2      # head pairs (one qT partition block each)
FCH = F // 128      # f contraction chunks
KCH = LK // 128     # lk chunks
NL = LK // LQ       # kv l-blocks

_CACHED = {}


def build_nc():
    nc = bacc.Bacc("TRN2", target_bir_lowering=False, debug=False,
                   num_devices=NCORES)
    dt_in = [
        ("xq_t", [FCH, 128, LQ], BF16),        # [f, p, lq]
        ("xkv_t", [NL, FCH, 128, LQ], BF16),   # [l, f, p, lq]
        ("mask_t", [KCH, 128, LQ], FP16),      # [c, p, lq]
        ("wq", [FCH, 128, FCH, 128], BF16),    # [j, p, f, m]
        ("wkv", [128, FCH, 128], BF16),        # [p, f, m]
        ("wo", [FCH, 128, FCH, 128], BF16),    # [fb, p, j, m]
        ("bqbo", [128, 2 * FCH], F32),         # cols 0:8 bq-blocks, 8:16 bo
        ("bkv", [2 * D], F32),
        ("cosq", [128, LQ], F32),
        ("sinq", [128, LQ], F32),
        ("cksk", [D, 2 * LK], F32),            # [p, (cos|sin)*lk]
    ]
    t = {name: nc.dram_tensor(name, shape, dt, kind="ExternalInput")
         for name, shape, dt in dt_in}
    yT = nc.dram_tensor("yT", [F, LQ], F32, kind="ExternalOutput")

    with tile.TileContext(nc) as tc:
        with (
            tc.tile_pool(name="persist", bufs=1) as persist,
            tc.tile_pool(name="ptiles", bufs=3) as ptp,
            tc.tile_pool(name="small", bufs=4) as small,
            tc.tile_pool(name="xin", bufs=2) as xin,
            tc.tile_pool(name="wst", bufs=2) as wst,
            tc.tile_pool(name="qraw", bufs=2) as qrp,
            tc.tile_pool(name="kvraw", bufs=2) as kvp,
            tc.tile_pool(name="ropetmp", bufs=2) as rtp,
            tc.tile_pool(name="rec", bufs=2) as recp,
            tc.tile_pool(name="yout", bufs=2) as yout,
            tc.tile_pool(name="psa", bufs=2, space="PSUM") as psa,   # 2 banks
            tc.tile_pool(name="psb", bufs=2, space="PSUM") as psb,   # 2 banks
            tc.tile_pool(name="psst", bufs=2, space="PSUM") as psst,  # 4 banks
        ):
            # ---------------- small constants (gpsimd DMA queue) ---------
            cq = persist.tile([128, LQ], F32)
            sq = persist.tile([128, LQ], F32)
            cksk = persist.tile([D, 2, LK], F32)
            nc.gpsimd.dma_start(cq, t["cosq"].ap())
            nc.gpsimd.dma_start(sq, t["sinq"].ap())
            nc.gpsimd.dma_start(cksk,
                                t["cksk"].ap().rearrange("p (a l) -> p a l", a=2))
            ck = cksk[:, 0, :]
            sk = cksk[:, 1, :]
            bqbo = small.tile([128, 2 * FCH], F32, tag="bias")
            nc.gpsimd.dma_start(bqbo, t["bqbo"].ap())
            bq_sb = bqbo[:, 0:FCH]
            bo_sb = bqbo[:, FCH:2 * FCH]
            bkv_sb = small.tile([128, 1], F32, tag="bias2")
            nc.gpsimd.dma_start(bkv_sb, t["bkv"].ap().unsqueeze(1))

            # mask chunks, duplicated per head-half so the pt multiply is a
            # single free-size-1024 DVE op (needed from attention start; 4MB)
            mt2 = persist.tile([128, KCH, 2, LQ], FP16)
            for c in range(KCH):
                for tt in range(2):
                    nc.gpsimd.dma_start(mt2[:, c, tt, :], t["mask_t"].ap()[c])

            # Wo prefetch (needed only in phase D; ~2MB)
            wo_sb = persist.tile([128, FCH, FCH, 128], BF16)
            for fb in range(FCH):
                nc.gpsimd.dma_start(wo_sb[:, fb], t["wo"].ap()[fb])

            # ---------------- persistent compute tiles -------------------
            qrot = persist.tile([128, PAIRS, LQ], BF16)
            ktop = persist.tile([128, LK], BF16)          # k rows 0:64
            kbot = persist.tile([128, LK], BF16)          # k rows 64:128
            vaug = persist.tile([128, KCH, D + 1], FP16)  # V chunks + ones col
            obig = persist.tile([128, PAIRS, LQ], BF16)   # normalized O^T

            idt = small.tile([128, 128], F32, tag="ident")
            make_identity(nc, idt)
            # halves-swap permutation matrix: M[p, p-xor-32-within-head] = 1
            swpf = small.tile([128, 128], F32, tag="swpf")
            nc.gpsimd.memset(swpf, 0.0)
            for o1, o2 in ((0, 32), (32, 0), (64, 96), (96, 64)):
                nc.gpsimd.affine_select(
                    out=swpf[o1:o1 + 32, o2:o2 + 32],
                    in_=swpf[o1:o1 + 32, o2:o2 + 32],
                    compare_op=mybir.AluOpType.not_equal, fill=1.0,
                    base=0, pattern=[[-1, 32]], channel_multiplier=1)
            swp = small.tile([128, 128], BF16, tag="swp")
            nc.vector.tensor_copy(swp, swpf)
            nc.vector.memset(ktop[64:128], 0.0)
            nc.vector.memset(kbot[0:64], 0.0)
            nc.vector.memset(vaug[:, :, D:D + 1], 1.0)

            # ================= phase A: KV chain per l-block ==============
            wkv_sb = wst.tile([128, FCH, 128], BF16, tag="wkv")
            nc.sync.dma_start(wkv_sb, t["wkv"].ap())
            for l in range(NL):
                xkv = xin.tile([128, FCH, LQ], BF16, tag="x")
                for f in range(FCH):
                    nc.sync.dma_start(xkv[:, f, :], t["xkv_t"].ap()[l, f])
                pkv = psb.tile([128, LQ], F32, tag="b")
                for f in range(FCH):
                    nc.tensor.matmul(pkv, wkv_sb[:, f, :], xkv[:, f, :],
                                     start=(f == 0), stop=(f == FCH - 1))
                kvl = kvp.tile([128, LQ], F32, tag="kv")
                nc.vector.tensor_scalar_add(kvl, pkv, bkv_sb[:, 0:1])

                # RoPE on k rows 0:64: krot = k*cos + Swap @ (k*sin_signed)
                lsl = ts(l, LQ)
                tmk = rtp.tile([D, LQ], BF16, tag="ksin")
                nc.vector.tensor_mul(tmk, kvl[0:64], sk[:, lsl])
                kc = rtp.tile([D, LQ], F32, tag="kcos")
                nc.vector.tensor_mul(kc, kvl[0:64], ck[:, lsl])
                pswk = psa.tile([128, LQ], F32, tag="a")
                nc.tensor.matmul(pswk[0:64], swp[0:64, 0:64], tmk,
                                 start=True, stop=True)
                nc.vector.tensor_add(ktop[0:64, lsl], kc, pswk[0:64])
                nc.gpsimd.dma_start(kbot[64:128, lsl], ktop[0:64, lsl])

                # V transpose into vaug chunks (+ copy on idle ACT engine)
                for ci in range(4):
                    c = 4 * l + ci
                    tp = psa.tile([128, LQ], F32, tag="a")
                    nc.tensor.transpose(tp[:, 0:64], kvl[64:128, ts(ci, 128)],
                                        idt[64:128, 64:128])
                    nc.scalar.copy(vaug[:, c, 0:D], tp[:, 0:64])

            # ---- Q projection + RoPE for one head-pair ----
            xq = persist.tile([128, FCH, LQ], BF16)
            for f in range(FCH):
                nc.sync.dma_start(xq[:, f, :], t["xq_t"].ap()[f])

            # All Q projections run before attention: keeps the per-pair
            # PSUM tag rotation at exactly one long-lived tile (oa) per
            # cycle, so cross-pair epilogue staging can't alias a live
            # accumulator bank.
            for j in range(PAIRS):
                wq_j = wst.tile([128, FCH, 128], BF16, tag="wq")
                nc.sync.dma_start(wq_j, t["wq"].ap()[j])
                psq = psa.tile([128, LQ], F32, tag="a")
                for f in range(FCH):
                    nc.tensor.matmul(psq, wq_j[:, f, :], xq[:, f, :],
                                     start=(f == 0), stop=(f == FCH - 1))
                qraw = qrp.tile([128, LQ], F32, tag="q")
                nc.vector.tensor_scalar_add(qraw, psq, bq_sb[:, j:j + 1])
                tmq = rtp.tile([128, LQ], BF16, tag="qsin")
                nc.vector.tensor_mul(tmq, qraw, sq)
                psw = psa.tile([128, LQ], F32, tag="a")
                nc.tensor.matmul(psw, swp, tmq, start=True, stop=True)
                qc = rtp.tile([128, LQ], F32, tag="qcos")
                nc.vector.tensor_mul(qc, qraw, cq)
                nc.vector.tensor_add(qrot[:, j, :], qc, psw)

            # ================= phase C: attention =================
            onesf = small.tile([1, D], F32, tag="onesf")
            nc.vector.memset(onesf, 1.0)

            def make_epilogue(j, oa, ob):
                """Normalize pair j's O accumulators. Returned as staged
                closures run inside pair j+1's chunk loop so nothing here
                sits on any engine's critical path. The reciprocal reads
                the PSUM denominator rows directly (f32), is bitcast to
                f32r for a K=1 ones-matmul broadcast down 64 partitions
                (rbp shares the st tag's PSUM buffers), then one DVE copy
                to SBUF feeds the two normalize muls."""
                den = recp.tile([1, 2, LQ], F32, tag="den")
                rcf = recp.tile([1, 2, LQ], F32, tag="rcf")
                rbs = recp.tile([D, 2, LQ], F32, tag="rbs")
                osb = recp.tile([D, LQ], BF16, tag="osb")
                state = {}

                def s_den():
                    # custom-DVE ops can't address PSUM; stage via SBUF
                    nc.vector.tensor_copy(den[:, 0, :], oa[D:D + 1, :])
                    nc.vector.tensor_copy(den[:, 1, :], ob[D:D + 1, :])

                def s_recip():
                    nc.vector.reciprocal_approx_fast(rcf, den)

                def s_bcast():
                    rbp = psst.tile([128, 2, LQ], F32, tag="st")
                    for tt in range(2):
                        nc.tensor.matmul(rbp[0:D, tt, :], onesf,
                                         rcf[0:1, tt, :],
                                         start=True, stop=True)
                    state["rbp"] = rbp

                def s_copy():
                    nc.vector.tensor_copy(rbs, state["rbp"][0:D, :, :])

                def s_mul_a():
                    nc.vector.tensor_mul(obig[0:D, j, :], oa[0:D, :],
                                         rbs[:, 0, :])

                def s_mul_b():
                    nc.vector.tensor_mul(osb, ob[0:D, :], rbs[:, 1, :])
                    nc.gpsimd.dma_start(obig[64:128, j, :], osb)

                return {0: s_den, 1: s_recip, 2: s_bcast, 3: s_copy,
                        5: s_mul_a, 6: s_mul_b}

            pend = {}
            for j in range(PAIRS):
                oa = psa.tile([128, LQ], F32, tag="a")
                ob = psb.tile([128, LQ], F32, tag="b")
                prev_pt = None

                def flush_o(c, pt, oa=oa, ob=ob):
                    nc.tensor.matmul(oa[0:D + 1, :], vaug[:, c, :],
                                     pt[:, 0, :], start=(c == 0),
                                     stop=(c == KCH - 1))
                    nc.tensor.matmul(ob[0:D + 1, :], vaug[:, c, :],
                                     pt[:, 1, :], start=(c == 0),
                                     stop=(c == KCH - 1))

                for c in range(KCH):
                    st = psst.tile([128, 2, LQ], F32, tag="st")
                    nc.tensor.matmul(st[:, 0, :], ktop[:, ts(c, 128)],
                                     qrot[:, j, :], start=True, stop=True)
                    nc.tensor.matmul(st[:, 1, :], kbot[:, ts(c, 128)],
                                     qrot[:, j, :], start=True, stop=True)
                    pt = ptp.tile([128, 2, LQ], FP16, tag="p")
                    nc.scalar.activation(pt, st,
                                         mybir.ActivationFunctionType.Exp)
                    nc.vector.tensor_mul(pt[:, :, :], pt[:, :, :],
                                         mt2[:, c, :, :])
                    # software pipeline: O for chunk c-1 goes behind the
                    # S matmuls of chunk c so the in-order PE queue never
                    # blocks on the exp->mask round-trip of the same chunk
                    if prev_pt is not None:
                        flush_o(c - 1, prev_pt)
                    prev_pt = pt
                    if c in pend:
                        pend.pop(c)()      # staged epilogue of pair j-1
                flush_o(KCH - 1, prev_pt)
                pend = make_epilogue(j, oa, ob)
            for c in sorted(pend):
                pend[c]()                  # last pair's epilogue

            # ================= phase D: output projection =================
            for fb in range(FCH):
                psy = psa.tile([128, LQ], F32, tag="a")
                for j in range(FCH):
                    nc.tensor.matmul(psy, wo_sb[:, fb, j, :], obig[:, j, :],
                                     start=(j == 0), stop=(j == FCH - 1))
                ysb = yout.tile([128, LQ], F32, tag="y")
                nc.vector.tensor_scalar_add(ysb, psy, bo_sb[:, fb:fb + 1])
                nc.sync.dma_start(yT.ap()[ts(fb, 128), :], ysb)

    nc.compile()
    return nc


def _tables():
    """RoPE tables in halves-permuted basis: rows i (even-half) hold +sin,
    rows 32+i (odd-half) hold -sin (for the tmp-then-swap formulation)."""
    inv_freq = 1.0 / (10000.0 ** (np.arange(0, D, 2, dtype=np.float64) / D))
    ang = np.outer(inv_freq, np.arange(L, dtype=np.float64))  # [32, L]
    cos = np.cos(ang).astype(np.float32)
    sin = np.sin(ang).astype(np.float32)
    cos64 = np.concatenate([cos, cos], axis=0)                # [64, L]
    sin_sgn = np.concatenate([sin, -sin], axis=0)             # [64, L]
    return cos64, sin_sgn


def _prep_weights(Wq, bq, Wk, bk, Wv, bv, Wo, bo):
    perm = np.concatenate([np.arange(0, D, 2), np.arange(1, D, 2)])
    WqP = np.asarray(Wq, dtype=np.float32)[:, :, perm].reshape(F, H * D)
    bqP = np.asarray(bq, dtype=np.float32)[:, perm].reshape(H * D)
    WkP = np.asarray(Wk, dtype=np.float32)[:, perm]
    bkP = np.asarray(bk, dtype=np.float32)[perm]
    Wkv = np.concatenate([WkP, np.asarray(Wv, dtype=np.float32)], axis=1)
    bkv = np.concatenate([bkP, np.asarray(bv, dtype=np.float32)])
    WoR = np.asarray(Wo, dtype=np.float32).reshape(H * D, F)
    bo_ = np.asarray(bo, dtype=np.float32)

    wq_pret = np.ascontiguousarray(
        WqP.reshape(FCH, 128, FCH, 128).transpose(2, 1, 0, 3)).astype(
            ml_dtypes.bfloat16)
    wkv_pret = np.ascontiguousarray(
        Wkv.reshape(FCH, 128, 128).transpose(1, 0, 2)).astype(
            ml_dtypes.bfloat16)
    wo_pret = np.ascontiguousarray(
        WoR.reshape(FCH, 128, FCH, 128).transpose(2, 1, 0, 3)).astype(
            ml_dtypes.bfloat16)
    bqbo = np.ascontiguousarray(np.concatenate(
        [bqP.reshape(FCH, 128).T, bo_.reshape(FCH, 128).T], axis=1))
    return wq_pret, wkv_pret, wo_pret, bqbo, bkv


def kernel(inputs_q, inputs_kv, mask, Wq, bq, Wk, bk, Wv, bv, Wo, bo):
    if "nc" not in _CACHED:
        _CACHED["nc"] = build_nc()
    nc = _CACHED["nc"]

    wq_pret, wkv_pret, wo_pret, bqbo, bkv = _prep_weights(
        Wq, bq, Wk, bk, Wv, bv, Wo, bo)

    cos64, sin_sgn = _tables()
    scale = 1.0 / np.sqrt(np.float32(D))
    cksk = np.ascontiguousarray(
        np.concatenate([cos64, sin_sgn], axis=1))      # [64, 2*L] (L=LK)
    cosq_full = np.tile(cos64 * scale, (2, 1))         # [128, L]
    sinq_full = np.tile(sin_sgn * scale, (2, 1))

    xq = np.asarray(inputs_q, dtype=np.float32)
    xkv = np.asarray(inputs_kv, dtype=np.float32)
    mk = np.asarray(mask)

    in_maps = []
    for core in range(NCORES):
        b = core // 4
        qs = (core % 4) * LQ
        xq_t = np.ascontiguousarray(
            xq[b, qs:qs + LQ, :].T.reshape(FCH, 128, LQ)).astype(
                ml_dtypes.bfloat16)
        xkv_t = np.ascontiguousarray(
            xkv[b].T.reshape(FCH, 128, NL, LQ).transpose(2, 0, 1, 3)).astype(
                ml_dtypes.bfloat16)
        mask_t = np.ascontiguousarray(
            mk[b, 0, qs:qs + LQ, :].T.reshape(KCH, 128, LQ)
            .astype(np.float16))
        in_maps.append({
            "xq_t": xq_t,
            "xkv_t": xkv_t,
            "mask_t": mask_t,
            "wq": wq_pret,
            "wkv": wkv_pret,
            "wo": wo_pret,
            "bqbo": bqbo,
            "bkv": bkv,
            "cosq": np.ascontiguousarray(cosq_full[:, qs:qs + LQ]),
            "sinq": np.ascontiguousarray(sinq_full[:, qs:qs + LQ]),
            "cksk": cksk,
        })

    res = bass_utils.run_bass_kernel_spmd(nc, in_maps,
                                          core_ids=list(range(NCORES)))
    _CACHED["last_results"] = res
    _CACHED["last_maps"] = in_maps

    out = np.empty((B, L, F), dtype=np.float32)
    for core in range(NCORES):
        b = core // 4
        qs = (core % 4) * LQ
        out[b, qs:qs + LQ, :] = res.results[core]["yT"].T
    return out


# revision 33
# speedup vs baseline: 1.1931x; 1.0475x over previous
"""MQA attention (B=2, Lq=Lkv=2048, F=1024, H=16, D=64) on 8 TRN2 cores.

Sharding: core = (batch, query-block-of-512). Each core computes its full
output rows (all 16 heads + output projection) -> no collectives; host
concatenates per-core yT slabs.

v2 dataflow (bf16/fp16 matmul operands, f32 PSUM accumulation):
  KV chain first (per 512-row l-block): kvT = Wkv.T @ xkvT -> RoPE-k
  (halves-permuted basis, swap via small PE matmul) -> ktop/kbot bf16;
  V transposed into vaug fp16 (ones col 64 = softmax denominator row).
  Q proj per head-pair j (interleaved into the attention pair loop):
  qT = Wq_j.T @ xqT -> RoPE -> qrot bf16.
  Attention per (pair j, kv-chunk c): S^T x2 (ktop/kbot stationary) ->
  exp on ACT ([128,2,512] PSUM supertile -> fp16) -> mask mul x2 on DVE
  (fp16 2x mode) -> O accumulation x2 (vaug stationary).
  Pair epilogue off the PE critical path: denominator rows -> DVE
  reciprocal_approx_fast -> fp16 -> K=1 ones matmul broadcast -> DVE
  normalize muls -> obig bf16 (head 2j+1 half moved down via gpsimd DMA).
  yT = Wo-chunks.T @ obig (+bo), Wo fully prefetched during attention.
"""

import ml_dtypes
import numpy as np

import concourse.bass as bass
import concourse.tile as tile
from concourse import bacc, mybir
from concourse import bass_utils
from concourse.bass import ts
from concourse.masks import make_identity

F32 = mybir.dt.float32
BF16 = mybir.dt.bfloat16
FP16 = mybir.dt.float16

B, L, F, H, D = 2, 2048, 1024, 16, 64
LQ = 512            # query rows per core
LK = 2048           # kv rows (full)
NCORES = 8
PAIRS = H // 2      # head pairs (one qT partition block each)
FCH = F // 128      # f contraction chunks
KCH = LK // 128     # lk chunks
NL = LK // LQ       # kv l-blocks

_CACHED = {}


def build_nc():
    nc = bacc.Bacc("TRN2", target_bir_lowering=False, debug=False,
                   num_devices=NCORES)
    dt_in = [
        ("xq_t", [FCH, 128, LQ], BF16),        # [f, p, lq]
        ("xkv_t", [NL, FCH, 128, LQ], BF16),   # [l, f, p, lq]
        ("mask_t", [KCH, 128, LQ], FP16),      # [c, p, lq]
        ("wq", [FCH, 128, FCH, 128], BF16),    # [j, p, f, m]
        ("wkv", [128, FCH, 128], BF16),        # [p, f, m]
        ("wo", [FCH, 128, FCH, 128], BF16),    # [fb, p, j, m]
        ("bqbo", [128, 2 * FCH], F32),         # cols 0:8 bq-blocks, 8:16 bo
        ("bkv", [2 * D], F32),
        ("cosq", [128, LQ], BF16),
        ("sinq", [128, LQ], BF16),
        ("cksk", [D, 2 * LK], BF16),           # [p, (cos|sin)*lk]
    ]
    t = {name: nc.dram_tensor(name, shape, dt, kind="ExternalInput")
         for name, shape, dt in dt_in}
    yT = nc.dram_tensor("yT", [F, LQ], F32, kind="ExternalOutput")

    with tile.TileContext(nc) as tc:
        with (
            tc.tile_pool(name="persist", bufs=1) as persist,
            tc.tile_pool(name="ptiles", bufs=3) as ptp,
            tc.tile_pool(name="small", bufs=4) as small,
            tc.tile_pool(name="xin", bufs=2) as xin,
            tc.tile_pool(name="wst", bufs=2) as wst,
            tc.tile_pool(name="qraw", bufs=2) as qrp,
            tc.tile_pool(name="kvraw", bufs=2) as kvp,
            tc.tile_pool(name="ropetmp", bufs=2) as rtp,
            tc.tile_pool(name="rec", bufs=2) as recp,
            tc.tile_pool(name="yout", bufs=2) as yout,
            tc.tile_pool(name="psa", bufs=2, space="PSUM") as psa,   # 2 banks
            tc.tile_pool(name="psb", bufs=2, space="PSUM") as psb,   # 2 banks
            tc.tile_pool(name="psst", bufs=2, space="PSUM") as psst,  # 4 banks
        ):
            # ---------------- small constants (gpsimd DMA queue) ---------
            cq = persist.tile([128, LQ], BF16)
            sq = persist.tile([128, LQ], BF16)
            cksk = persist.tile([D, 2, LK], BF16)
            nc.gpsimd.dma_start(cq, t["cosq"].ap())
            nc.gpsimd.dma_start(sq, t["sinq"].ap())
            nc.gpsimd.dma_start(cksk,
                                t["cksk"].ap().rearrange("p (a l) -> p a l", a=2))
            ck = cksk[:, 0, :]
            sk = cksk[:, 1, :]
            bqbo = small.tile([128, 2 * FCH], F32, tag="bias")
            nc.gpsimd.dma_start(bqbo, t["bqbo"].ap())
            bq_sb = bqbo[:, 0:FCH]
            bo_sb = bqbo[:, FCH:2 * FCH]
            bkv_sb = small.tile([128, 1], F32, tag="bias2")
            nc.gpsimd.dma_start(bkv_sb, t["bkv"].ap().unsqueeze(1))

            # mask chunks, duplicated per head-half so the pt multiply is a
            # single free-size-1024 DVE op. DMAs are issued lazily inside
            # the attention loop so they never contend with the critical
            # xkv/xq/wq input stream.
            mt2 = persist.tile([128, KCH, 2, LQ], FP16)

            # ---------------- persistent compute tiles -------------------
            qrot = persist.tile([128, PAIRS, LQ], BF16)
            ktop = persist.tile([128, LK], BF16)          # k rows 0:64
            kbot = persist.tile([128, LK], BF16)          # k rows 64:128
            vaug = persist.tile([128, KCH, D + 1], FP16)  # V chunks + ones col
            obig = persist.tile([128, PAIRS, LQ], BF16)   # normalized O^T

            idt = small.tile([128, 128], F32, tag="ident")
            make_identity(nc, idt)
            # halves-swap permutation matrix: M[p, p-xor-32-within-head] = 1
            swpf = small.tile([128, 128], F32, tag="swpf")
            nc.gpsimd.memset(swpf, 0.0)
            for o1, o2 in ((0, 32), (32, 0), (64, 96), (96, 64)):
                nc.gpsimd.affine_select(
                    out=swpf[o1:o1 + 32, o2:o2 + 32],
                    in_=swpf[o1:o1 + 32, o2:o2 + 32],
                    compare_op=mybir.AluOpType.not_equal, fill=1.0,
                    base=0, pattern=[[-1, 32]], channel_multiplier=1)
            swp = small.tile([128, 128], BF16, tag="swp")
            nc.vector.tensor_copy(swp, swpf)
            nc.vector.memset(ktop[64:128], 0.0)
            nc.vector.memset(kbot[0:64], 0.0)
            nc.vector.memset(vaug[:, :, D:D + 1], 1.0)

            # ======== phase A/B: KV chain + Q projections, interleaved ====
            # Sync-queue DMA order IS the bandwidth priority order: wkv,
            # xkv_l0, xq, wq0, xkv_l1, wq1, ... One batched DMA per block
            # (DMA issue on the queue engine costs ~600ns per instruction).
            wkv_sb = wst.tile([128, FCH, 128], BF16, tag="wkv")
            nc.sync.dma_start(wkv_sb, t["wkv"].ap())
            xq = persist.tile([128, FCH, LQ], BF16)
            xkvs = []
            for l in range(NL):
                xkv = xin.tile([128, FCH, LQ], BF16, tag="x", bufs=NL)
                nc.sync.dma_start(
                    xkv, t["xkv_t"].ap()[l].rearrange("f p lq -> p f lq"))
                xkvs.append(xkv)
                if l == 0:
                    nc.sync.dma_start(
                        xq, t["xq_t"].ap().rearrange("f p lq -> p f lq"))
            wqs = []
            for j in range(PAIRS):
                wq_j = wst.tile([128, FCH, 128], BF16, tag="wq", bufs=8)
                nc.sync.dma_start(wq_j, t["wq"].ap()[j])
                wqs.append(wq_j)

            def kv_block(l):
                xkv = xkvs[l]
                pkv = psb.tile([128, LQ], F32, tag="b")
                for f in range(FCH):
                    nc.tensor.matmul(pkv, wkv_sb[:, f, :], xkv[:, f, :],
                                     start=(f == 0), stop=(f == FCH - 1))
                kvl = kvp.tile([128, LQ], F32, tag="kv")
                nc.vector.tensor_scalar_add(kvl, pkv, bkv_sb[:, 0:1])

                # RoPE on k rows 0:64: krot = k*cos + Swap @ (k*sin_signed)
                lsl = ts(l, LQ)
                tmk = rtp.tile([D, LQ], BF16, tag="ksin")
                nc.vector.tensor_mul(tmk, kvl[0:64], sk[:, lsl])
                kc = rtp.tile([D, LQ], F32, tag="kcos")
                nc.vector.tensor_mul(kc, kvl[0:64], ck[:, lsl])
                pswk = psa.tile([128, LQ], F32, tag="a")
                nc.tensor.matmul(pswk[0:64], swp[0:64, 0:64], tmk,
                                 start=True, stop=True)
                nc.vector.tensor_add(ktop[0:64, lsl], kc, pswk[0:64])
                nc.gpsimd.dma_start(kbot[64:128, lsl], ktop[0:64, lsl])

                # V transpose into vaug chunks (+ copy on idle ACT engine)
                for ci in range(4):
                    c = 4 * l + ci
                    tp = psa.tile([128, LQ], F32, tag="a")
                    nc.tensor.transpose(tp[:, 0:64], kvl[64:128, ts(ci, 128)],
                                        idt[64:128, 64:128])
                    nc.scalar.copy(vaug[:, c, 0:D], tp[:, 0:64])

            def q_proj(j):
                psq = psa.tile([128, LQ], F32, tag="a")
                for f in range(FCH):
                    nc.tensor.matmul(psq, wqs[j][:, f, :], xq[:, f, :],
                                     start=(f == 0), stop=(f == FCH - 1))
                qraw = qrp.tile([128, LQ], F32, tag="q")
                nc.vector.tensor_scalar_add(qraw, psq, bq_sb[:, j:j + 1])
                tmq = rtp.tile([128, LQ], BF16, tag="qsin")
                nc.vector.tensor_mul(tmq, qraw, sq)
                psw = psa.tile([128, LQ], F32, tag="a")
                nc.tensor.matmul(psw, swp, tmq, start=True, stop=True)
                qc = rtp.tile([128, LQ], F32, tag="qcos")
                nc.vector.tensor_mul(qc, qraw, cq)
                nc.vector.tensor_add(qrot[:, j, :], qc, psw)

            # All Q projections run before attention: keeps the per-pair
            # PSUM tag rotation at exactly one long-lived tile (oa) per
            # cycle, so cross-pair epilogue staging can't alias a live
            # accumulator bank. PE emission follows DMA arrival order.
            for l in range(NL):
                kv_block(l)
                q_proj(l)
            for j in range(NL, PAIRS):
                q_proj(j)

            # ================= phase C: attention =================
            onesf = small.tile([1, D], F32, tag="onesf")
            nc.vector.memset(onesf, 1.0)

            def make_epilogue(j, oa, ob):
                """Normalize pair j's O accumulators. Returned as staged
                closures run inside pair j+1's chunk loop so nothing here
                sits on any engine's critical path. The reciprocal reads
                the PSUM denominator rows directly (f32), is bitcast to
                f32r for a K=1 ones-matmul broadcast down 64 partitions
                (rbp shares the st tag's PSUM buffers), then one DVE copy
                to SBUF feeds the two normalize muls."""
                den = recp.tile([1, 2, LQ], F32, tag="den")
                rcf = recp.tile([1, 2, LQ], F32, tag="rcf")
                rbs = recp.tile([D, 2, LQ], F32, tag="rbs")
                osb = recp.tile([D, LQ], BF16, tag="osb")
                state = {}

                def s_den():
                    # custom-DVE ops can't address PSUM; stage via SBUF
                    nc.vector.tensor_copy(den[:, 0, :], oa[D:D + 1, :])
                    nc.vector.tensor_copy(den[:, 1, :], ob[D:D + 1, :])

                def s_recip():
                    nc.vector.reciprocal_approx_fast(rcf, den)

                def s_bcast():
                    rbp = psst.tile([128, 2, LQ], F32, tag="st")
                    for tt in range(2):
                        nc.tensor.matmul(rbp[0:D, tt, :], onesf,
                                         rcf[0:1, tt, :],
                                         start=True, stop=True)
                    state["rbp"] = rbp

                def s_copy():
                    nc.vector.tensor_copy(rbs, state["rbp"][0:D, :, :])

                def s_mul_a():
                    nc.vector.tensor_mul(obig[0:D, j, :], oa[0:D, :],
                                         rbs[:, 0, :])

                def s_mul_b():
                    nc.vector.tensor_mul(osb, ob[0:D, :], rbs[:, 1, :])
                    nc.gpsimd.dma_start(obig[64:128, j, :], osb)

                return {0: s_den, 1: s_recip, 2: s_bcast, 3: s_copy,
                        5: s_mul_a, 6: s_mul_b}

            # Flat (pair, chunk) stream with the S matmuls running one
            # chunk ahead of the O matmuls — continuous across pair
            # boundaries, so the in-order PE queue never drains behind
            # the exp->mask round-trip and the p-state stays ramped.
            def emit_s(j, c):
                st = psst.tile([128, 2, LQ], F32, tag="st")
                nc.tensor.matmul(st[:, 0, :], ktop[:, ts(c, 128)],
                                 qrot[:, j, :], start=True, stop=True)
                nc.tensor.matmul(st[:, 1, :], kbot[:, ts(c, 128)],
                                 qrot[:, j, :], start=True, stop=True)
                return st

            seq = [(j, c) for j in range(PAIRS) for c in range(KCH)]
            sts = {seq[0]: emit_s(*seq[0])}
            oab = {}
            pend = {}
            for i, (j, c) in enumerate(seq):
                if c == 0:
                    oab[j] = (psa.tile([128, LQ], F32, tag="a", name="oa"),
                              psb.tile([128, LQ], F32, tag="b", name="ob"))

                if j == 0:
                    for cm in ([0, 1, 2] if c == 0 else
                               [c + 2] if c + 2 < KCH else []):
                        for tt in range(2):
                            nc.sync.dma_start(mt2[:, cm, tt, :],
                                              t["mask_t"].ap()[cm])
                if i + 1 < len(seq):
                    sts[seq[i + 1]] = emit_s(*seq[i + 1])
                st = sts.pop((j, c))
                pt = ptp.tile([128, 2, LQ], FP16, tag="p")
                nc.scalar.activation(pt, st,
                                     mybir.ActivationFunctionType.Exp)
                nc.vector.tensor_mul(pt[:, :, :], pt[:, :, :],
                                     mt2[:, c, :, :])
                oa, ob = oab[j]
                nc.tensor.matmul(oa[0:D + 1, :], vaug[:, c, :],
                                 pt[:, 0, :], start=(c == 0),
                                 stop=(c == KCH - 1))
                nc.tensor.matmul(ob[0:D + 1, :], vaug[:, c, :],
                                 pt[:, 1, :], start=(c == 0),
                                 stop=(c == KCH - 1))
                if c in pend:
                    pend.pop(c)()          # staged epilogue of pair j-1
                if c == KCH - 1:
                    pend = make_epilogue(j, oa, ob)
                    oab.pop(j)
            for c in sorted(pend):
                pend[c]()                  # last pair's epilogue

            # ================= phase D: output projection =================
            wos = []
            for fb in range(FCH):
                wo_fb = wst.tile([128, FCH, 128], BF16, tag="wo", bufs=3,
                                 name="wo_fb")
                nc.gpsimd.dma_start(wo_fb, t["wo"].ap()[fb])
                wos.append(wo_fb)
            for fb in range(FCH):
                psy = psa.tile([128, LQ], F32, tag="a")
                for j in range(FCH):
                    nc.tensor.matmul(psy, wos[fb][:, j, :], obig[:, j, :],
                                     start=(j == 0), stop=(j == FCH - 1))
                ysb = yout.tile([128, LQ], F32, tag="y")
                nc.vector.tensor_scalar_add(ysb, psy, bo_sb[:, fb:fb + 1])
                nc.sync.dma_start(yT.ap()[ts(fb, 128), :], ysb)

    nc.compile()
    return nc


def _tables():
    """RoPE tables in halves-permuted basis: rows i (even-half) hold +sin,
    rows 32+i (odd-half) hold -sin (for the tmp-then-swap formulation)."""
    inv_freq = 1.0 / (10000.0 ** (np.arange(0, D, 2, dtype=np.float64) / D))
    ang = np.outer(inv_freq, np.arange(L, dtype=np.float64))  # [32, L]
    cos = np.cos(ang).astype(np.float32)
    sin = np.sin(ang).astype(np.float32)
    cos64 = np.concatenate([cos, cos], axis=0)                # [64, L]
    sin_sgn = np.concatenate([sin, -sin], axis=0)             # [64, L]
    return cos64, sin_sgn


def _prep_weights(Wq, bq, Wk, bk, Wv, bv, Wo, bo):
    perm = np.concatenate([np.arange(0, D, 2), np.arange(1, D, 2)])
    WqP = np.asarray(Wq, dtype=np.float32)[:, :, perm].reshape(F, H * D)
    bqP = np.asarray(bq, dtype=np.float32)[:, perm].reshape(H * D)
    WkP = np.asarray(Wk, dtype=np.float32)[:, perm]
    bkP = np.asarray(bk, dtype=np.float32)[perm]
    Wkv = np.concatenate([WkP, np.asarray(Wv, dtype=np.float32)], axis=1)
    bkv = np.concatenate([bkP, np.asarray(bv, dtype=np.float32)])
    WoR = np.asarray(Wo, dtype=np.float32).reshape(H * D, F)
    bo_ = np.asarray(bo, dtype=np.float32)

    wq_pret = np.ascontiguousarray(
        WqP.reshape(FCH, 128, FCH, 128).transpose(2, 1, 0, 3)).astype(
            ml_dtypes.bfloat16)
    wkv_pret = np.ascontiguousarray(
        Wkv.reshape(FCH, 128, 128).transpose(1, 0, 2)).astype(
            ml_dtypes.bfloat16)
    wo_pret = np.ascontiguousarray(
        WoR.reshape(FCH, 128, FCH, 128).transpose(2, 1, 0, 3)).astype(
            ml_dtypes.bfloat16)
    bqbo = np.ascontiguousarray(np.concatenate(
        [bqP.reshape(FCH, 128).T, bo_.reshape(FCH, 128).T], axis=1))
    return wq_pret, wkv_pret, wo_pret, bqbo, bkv


def kernel(inputs_q, inputs_kv, mask, Wq, bq, Wk, bk, Wv, bv, Wo, bo):
    if "nc" not in _CACHED:
        _CACHED["nc"] = build_nc()
    nc = _CACHED["nc"]

    wq_pret, wkv_pret, wo_pret, bqbo, bkv = _prep_weights(
        Wq, bq, Wk, bk, Wv, bv, Wo, bo)

    cos64, sin_sgn = _tables()
    scale = 1.0 / np.sqrt(np.float32(D))
    cksk = np.ascontiguousarray(
        np.concatenate([cos64, sin_sgn], axis=1))      # [64, 2*L] (L=LK)
    cosq_full = np.tile(cos64 * scale, (2, 1))         # [128, L]
    sinq_full = np.tile(sin_sgn * scale, (2, 1))

    xq = np.asarray(inputs_q, dtype=np.float32)
    xkv = np.asarray(inputs_kv, dtype=np.float32)
    mk = np.asarray(mask)

    in_maps = []
    for core in range(NCORES):
        b = core // 4
        qs = (core % 4) * LQ
        xq_t = np.ascontiguousarray(
            xq[b, qs:qs + LQ, :].T.reshape(FCH, 128, LQ)).astype(
                ml_dtypes.bfloat16)
        xkv_t = np.ascontiguousarray(
            xkv[b].T.reshape(FCH, 128, NL, LQ).transpose(2, 0, 1, 3)).astype(
                ml_dtypes.bfloat16)
        mask_t = np.ascontiguousarray(
            mk[b, 0, qs:qs + LQ, :].T.reshape(KCH, 128, LQ)
            .astype(np.float16))
        in_maps.append({
            "xq_t": xq_t,
            "xkv_t": xkv_t,
            "mask_t": mask_t,
            "wq": wq_pret,
            "wkv": wkv_pret,
            "wo": wo_pret,
            "bqbo": bqbo,
            "bkv": bkv,
            "cosq": np.ascontiguousarray(
                cosq_full[:, qs:qs + LQ]).astype(ml_dtypes.bfloat16),
            "sinq": np.ascontiguousarray(
                sinq_full[:, qs:qs + LQ]).astype(ml_dtypes.bfloat16),
            "cksk": cksk.astype(ml_dtypes.bfloat16),
        })

    res = bass_utils.run_bass_kernel_spmd(nc, in_maps,
                                          core_ids=list(range(NCORES)))
    _CACHED["last_results"] = res
    _CACHED["last_maps"] = in_maps

    out = np.empty((B, L, F), dtype=np.float32)
    for core in range(NCORES):
        b = core // 4
        qs = (core % 4) * LQ
        out[b, qs:qs + LQ, :] = res.results[core]["yT"].T
    return out


# revision 37
# speedup vs baseline: 1.3107x; 1.0985x over previous
"""MQA attention (B=2, Lq=Lkv=2048, F=1024, H=16, D=64) on 8 TRN2 cores.

Sharding: core = (batch, query-block-of-512). Each core computes its full
output rows (all 16 heads + output projection) -> no collectives; host
concatenates per-core yT slabs.

v2 dataflow (bf16/fp16 matmul operands, f32 PSUM accumulation):
  KV chain first (per 512-row l-block): kvT = Wkv.T @ xkvT -> RoPE-k
  (halves-permuted basis, swap via small PE matmul) -> ktop/kbot bf16;
  V transposed into vaug fp16 (ones col 64 = softmax denominator row).
  Q proj per head-pair j (interleaved into the attention pair loop):
  qT = Wq_j.T @ xqT -> RoPE -> qrot bf16.
  Attention per (pair j, kv-chunk c): S^T x2 (ktop/kbot stationary) ->
  exp on ACT ([128,2,512] PSUM supertile -> fp16) -> mask mul x2 on DVE
  (fp16 2x mode) -> O accumulation x2 (vaug stationary).
  Pair epilogue off the PE critical path: denominator rows -> DVE
  reciprocal_approx_fast -> fp16 -> K=1 ones matmul broadcast -> DVE
  normalize muls -> obig bf16 (head 2j+1 half moved down via gpsimd DMA).
  yT = Wo-chunks.T @ obig (+bo), Wo fully prefetched during attention.
"""

import ml_dtypes
import numpy as np

import concourse.bass as bass
import concourse.tile as tile
from concourse import bacc, mybir
from concourse import bass_utils
from concourse.bass import ts
from concourse.masks import make_identity

F32 = mybir.dt.float32
BF16 = mybir.dt.bfloat16
FP16 = mybir.dt.float16

B, L, F, H, D = 2, 2048, 1024, 16, 64
LQ = 512            # query rows per core
LK = 2048           # kv rows (full)
NCORES = 8
PAIRS = H // 2      # head pairs (one qT partition block each)
FCH = F // 128      # f contraction chunks
KCH = LK // 128     # lk chunks
NL = LK // LQ       # kv l-blocks

_CACHED = {}


def build_nc():
    nc = bacc.Bacc("TRN2", target_bir_lowering=False, debug=False,
                   num_devices=NCORES)
    dt_in = [
        ("xq_t", [FCH, 128, LQ], BF16),        # [f, p, lq]
        ("xkv_t", [NL, FCH, 128, LQ], BF16),   # [l, f, p, lq]
        ("mask_t", [KCH, 128, LQ], FP16),      # [c, p, lq]
        ("wq", [FCH, 128, FCH, 128], BF16),    # [j, p, f, m]
        ("wkv", [128, FCH, 128], BF16),        # [p, f, m]
        ("wo", [FCH, 128, FCH, 128], BF16),    # [fb, p, j, m]
        ("bqbo", [128, 2 * FCH], F32),         # cols 0:8 bq-blocks, 8:16 bo
        ("bkv", [2 * D], F32),
        ("cosq", [128, LQ], BF16),
        ("sinq", [128, LQ], BF16),
        ("cksk", [D, 2 * LK], BF16),           # [p, (cos|sin)*lk]
    ]
    t = {name: nc.dram_tensor(name, shape, dt, kind="ExternalInput")
         for name, shape, dt in dt_in}
    yT = nc.dram_tensor("yT", [F, LQ], F32, kind="ExternalOutput")

    with tile.TileContext(nc) as tc:
        with (
            tc.tile_pool(name="persist", bufs=1) as persist,
            tc.tile_pool(name="ptiles", bufs=3) as ptp,
            tc.tile_pool(name="small", bufs=4) as small,
            tc.tile_pool(name="xin", bufs=2) as xin,
            tc.tile_pool(name="wst", bufs=2) as wst,
            tc.tile_pool(name="qraw", bufs=2) as qrp,
            tc.tile_pool(name="kvraw", bufs=2) as kvp,
            tc.tile_pool(name="ropetmp", bufs=2) as rtp,
            tc.tile_pool(name="rec", bufs=2) as recp,
            tc.tile_pool(name="yout", bufs=2) as yout,
            tc.tile_pool(name="psa", bufs=2, space="PSUM") as psa,   # 2 banks
            tc.tile_pool(name="psb", bufs=2, space="PSUM") as psb,   # 2 banks
            tc.tile_pool(name="psst", bufs=2, space="PSUM") as psst,  # 4 banks
        ):
            # ---------------- small constants (gpsimd DMA queue) ---------
            cq = persist.tile([128, LQ], BF16)
            sq = persist.tile([128, LQ], BF16)
            cksk = persist.tile([D, 2, LK], BF16)
            nc.gpsimd.dma_start(cq, t["cosq"].ap())
            nc.gpsimd.dma_start(sq, t["sinq"].ap())
            nc.gpsimd.dma_start(cksk,
                                t["cksk"].ap().rearrange("p (a l) -> p a l", a=2))
            ck = cksk[:, 0, :]
            sk = cksk[:, 1, :]
            bqbo = small.tile([128, 2 * FCH], F32, tag="bias")
            nc.gpsimd.dma_start(bqbo, t["bqbo"].ap())
            bq_sb = bqbo[:, 0:FCH]
            bo_sb = bqbo[:, FCH:2 * FCH]
            bkv_sb = small.tile([128, 1], F32, tag="bias2")
            nc.gpsimd.dma_start(bkv_sb, t["bkv"].ap().unsqueeze(1))

            # mask chunks, duplicated per head-half so the pt multiply is a
            # single free-size-1024 DVE op. DMAs are issued lazily inside
            # the attention loop so they never contend with the critical
            # xkv/xq/wq input stream.
            mt2 = persist.tile([128, KCH, 2, LQ], FP16)

            # ---------------- persistent compute tiles -------------------
            qrot = persist.tile([128, PAIRS, LQ], BF16)
            ktop = persist.tile([128, LK], BF16)          # k rows 0:64
            kbot = persist.tile([128, LK], BF16)          # k rows 64:128
            vaug = persist.tile([128, KCH, D + 1], FP16)  # V chunks + ones col
            obig = persist.tile([128, PAIRS, LQ], BF16)   # normalized O^T

            idt = small.tile([128, 128], F32, tag="ident")
            make_identity(nc, idt)
            # halves-swap permutation matrix: M[p, p-xor-32-within-head] = 1
            swpf = small.tile([128, 128], F32, tag="swpf")
            nc.gpsimd.memset(swpf, 0.0)
            for o1, o2 in ((0, 32), (32, 0), (64, 96), (96, 64)):
                nc.gpsimd.affine_select(
                    out=swpf[o1:o1 + 32, o2:o2 + 32],
                    in_=swpf[o1:o1 + 32, o2:o2 + 32],
                    compare_op=mybir.AluOpType.not_equal, fill=1.0,
                    base=0, pattern=[[-1, 32]], channel_multiplier=1)
            swp = small.tile([128, 128], BF16, tag="swp")
            nc.vector.tensor_copy(swp, swpf)
            nc.vector.memset(ktop[64:128], 0.0)
            nc.vector.memset(kbot[0:64], 0.0)
            nc.vector.memset(vaug[:, :, D:D + 1], 1.0)

            # ======== phase A/B: KV chain + Q projections, interleaved ====
            # Sync-queue DMA order IS the bandwidth priority order: wkv,
            # xkv_l0, xq, wq0, xkv_l1, wq1, ... One batched DMA per block
            # (DMA issue on the queue engine costs ~600ns per instruction).
            wkv_sb = wst.tile([128, FCH, 128], BF16, tag="wkv")
            nc.sync.dma_start(wkv_sb, t["wkv"].ap())
            xq = persist.tile([128, FCH, LQ], BF16)
            xkvs = []
            for l in range(NL):
                xkv = xin.tile([128, FCH, LQ], BF16, tag="x", bufs=NL)
                nc.sync.dma_start(
                    xkv, t["xkv_t"].ap()[l].rearrange("f p lq -> p f lq"))
                xkvs.append(xkv)
                if l == 0:
                    nc.sync.dma_start(
                        xq, t["xq_t"].ap().rearrange("f p lq -> p f lq"))
            wqs = []
            for j in range(PAIRS):
                wq_j = wst.tile([128, FCH, 128], BF16, tag="wq", bufs=8)
                nc.sync.dma_start(wq_j, t["wq"].ap()[j])
                wqs.append(wq_j)
                if j == 4:
                    # mask stream: quarters keep dependency granularity so
                    # pair-0 chunk c only waits on its own quarter; placed
                    # here so attention start (~23us) stays fed while late
                    # wq blocks (not needed before pair 5) yield bandwidth
                    for mq in range(4):
                        for tt in range(2):
                            nc.sync.dma_start(
                                mt2[:, 4 * mq:4 * mq + 4, tt, :],
                                t["mask_t"].ap()[4 * mq:4 * mq + 4]
                                .rearrange("c p lq -> p c lq"))

            def kv_block(l):
                xkv = xkvs[l]
                pkv = psb.tile([128, LQ], F32, tag="b")
                for f in range(FCH):
                    nc.tensor.matmul(pkv, wkv_sb[:, f, :], xkv[:, f, :],
                                     start=(f == 0), stop=(f == FCH - 1))
                kvl = kvp.tile([128, LQ], F32, tag="kv")
                nc.vector.tensor_scalar_add(kvl, pkv, bkv_sb[:, 0:1])

                # RoPE on k rows 0:64: krot = k*cos + Swap @ (k*sin_signed)
                lsl = ts(l, LQ)
                tmk = rtp.tile([D, LQ], BF16, tag="ksin")
                nc.vector.tensor_mul(tmk, kvl[0:64], sk[:, lsl])
                kc = rtp.tile([D, LQ], F32, tag="kcos")
                nc.vector.tensor_mul(kc, kvl[0:64], ck[:, lsl])
                pswk = psa.tile([128, LQ], F32, tag="a")
                nc.tensor.matmul(pswk[0:64], swp[0:64, 0:64], tmk,
                                 start=True, stop=True)
                nc.vector.tensor_add(ktop[0:64, lsl], kc, pswk[0:64])
                nc.gpsimd.dma_start(kbot[64:128, lsl], ktop[0:64, lsl])

                # V transpose into vaug chunks (+ copy on idle ACT engine)
                for ci in range(4):
                    c = 4 * l + ci
                    tp = psa.tile([128, LQ], F32, tag="a")
                    nc.tensor.transpose(tp[:, 0:64], kvl[64:128, ts(ci, 128)],
                                        idt[64:128, 64:128])
                    nc.scalar.copy(vaug[:, c, 0:D], tp[:, 0:64])

            def q_proj(j):
                psq = psa.tile([128, LQ], F32, tag="a")
                for f in range(FCH):
                    nc.tensor.matmul(psq, wqs[j][:, f, :], xq[:, f, :],
                                     start=(f == 0), stop=(f == FCH - 1))
                qraw = qrp.tile([128, LQ], F32, tag="q")
                nc.vector.tensor_scalar_add(qraw, psq, bq_sb[:, j:j + 1])
                tmq = rtp.tile([128, LQ], BF16, tag="qsin")
                nc.vector.tensor_mul(tmq, qraw, sq)
                psw = psa.tile([128, LQ], F32, tag="a")
                nc.tensor.matmul(psw, swp, tmq, start=True, stop=True)
                qc = rtp.tile([128, LQ], F32, tag="qcos")
                nc.vector.tensor_mul(qc, qraw, cq)
                nc.vector.tensor_add(qrot[:, j, :], qc, psw)

            # All Q projections run before attention: keeps the per-pair
            # PSUM tag rotation at exactly one long-lived tile (oa) per
            # cycle, so cross-pair epilogue staging can't alias a live
            # accumulator bank. PE emission follows DMA arrival order.
            for l in range(NL):
                kv_block(l)
                q_proj(l)
            for j in range(NL, PAIRS):
                q_proj(j)

            # ================= phase C: attention =================
            onesh = small.tile([1, D], FP16, tag="onesh")
            nc.vector.memset(onesh, 1.0)

            def make_epilogue(j, oa, ob):
                """Normalize pair j's O accumulators. Returned as staged
                closures run inside pair j+1's chunk loop so nothing here
                sits on any engine's critical path. The reciprocal reads
                the PSUM denominator rows directly (f32), is bitcast to
                f32r for a K=1 ones-matmul broadcast down 64 partitions
                (rbp shares the st tag's PSUM buffers), then one DVE copy
                to SBUF feeds the two normalize muls."""
                den = recp.tile([1, 2, LQ], F32, tag="den")
                rcf = recp.tile([1, 2, LQ], F32, tag="rcf")
                rch = recp.tile([1, 2, LQ], FP16, tag="rch")
                rbs = recp.tile([D, 2, LQ], FP16, tag="rbs")
                osb = recp.tile([D, LQ], BF16, tag="osb")
                state = {}

                def s_den():
                    # custom-DVE ops can't address PSUM; stage via SBUF
                    nc.vector.tensor_copy(den[:, 0, :], oa[D:D + 1, :])
                    nc.vector.tensor_copy(den[:, 1, :], ob[D:D + 1, :])

                def s_recip():
                    nc.vector.reciprocal_approx_fast(rcf, den)

                def s_cast():
                    # fp16 so the broadcast matmul streams at 1 cyc/row
                    # (an fp32 matmul lowers to TWO half-rate PE passes)
                    nc.vector.tensor_copy(rch, rcf)

                def s_bcast():
                    rbp = psst.tile([128, 2, LQ], F32, tag="st")
                    for tt in range(2):
                        nc.tensor.matmul(rbp[0:D, tt, :], onesh,
                                         rch[0:1, tt, :],
                                         start=True, stop=True)
                    state["rbp"] = rbp

                def s_copy():
                    nc.vector.tensor_copy(rbs, state["rbp"][0:D, :, :])

                def s_mul_a():
                    nc.vector.tensor_mul(obig[0:D, j, :], oa[0:D, :],
                                         rbs[:, 0, :])

                def s_mul_b():
                    nc.vector.tensor_mul(osb, ob[0:D, :], rbs[:, 1, :])
                    nc.gpsimd.dma_start(obig[64:128, j, :], osb)

                return {0: s_den, 1: s_recip, 2: s_cast, 3: s_bcast,
                        4: s_copy, 6: s_mul_a, 7: s_mul_b}

            # Flat (pair, chunk) stream with the S matmuls running one
            # chunk ahead of the O matmuls — continuous across pair
            # boundaries, so the in-order PE queue never drains behind
            # the exp->mask round-trip and the p-state stays ramped.
            def emit_s(j, c):
                st = psst.tile([128, 2, LQ], F32, tag="st")
                nc.tensor.matmul(st[:, 0, :], ktop[:, ts(c, 128)],
                                 qrot[:, j, :], start=True, stop=True)
                nc.tensor.matmul(st[:, 1, :], kbot[:, ts(c, 128)],
                                 qrot[:, j, :], start=True, stop=True)
                return st

            seq = [(j, c) for j in range(PAIRS) for c in range(KCH)]
            sts = {seq[0]: emit_s(*seq[0])}
            oab = {}
            pend = {}
            for i, (j, c) in enumerate(seq):
                if c == 0:
                    oab[j] = (psa.tile([128, LQ], F32, tag="a", name="oa"),
                              psb.tile([128, LQ], F32, tag="b", name="ob"))

                if i + 1 < len(seq):
                    sts[seq[i + 1]] = emit_s(*seq[i + 1])
                st = sts.pop((j, c))
                pt = ptp.tile([128, 2, LQ], FP16, tag="p")
                nc.scalar.activation(pt, st,
                                     mybir.ActivationFunctionType.Exp)
                nc.vector.tensor_mul(pt[:, :, :], pt[:, :, :],
                                     mt2[:, c, :, :])
                oa, ob = oab[j]
                nc.tensor.matmul(oa[0:D + 1, :], vaug[:, c, :],
                                 pt[:, 0, :], start=(c == 0),
                                 stop=(c == KCH - 1))
                nc.tensor.matmul(ob[0:D + 1, :], vaug[:, c, :],
                                 pt[:, 1, :], start=(c == 0),
                                 stop=(c == KCH - 1))
                if c in pend:
                    pend.pop(c)()          # staged epilogue of pair j-1
                if c == KCH - 1:
                    pend = make_epilogue(j, oa, ob)
                    oab.pop(j)
            for c in sorted(pend):
                pend[c]()                  # last pair's epilogue

            # ================= phase D: output projection =================
            wos = []
            for fb in range(FCH):
                wo_fb = wst.tile([128, FCH, 128], BF16, tag="wo", bufs=3,
                                 name="wo_fb")
                nc.gpsimd.dma_start(wo_fb, t["wo"].ap()[fb])
                wos.append(wo_fb)
            for fb in range(FCH):
                psy = psa.tile([128, LQ], F32, tag="a")
                for j in range(FCH):
                    nc.tensor.matmul(psy, wos[fb][:, j, :], obig[:, j, :],
                                     start=(j == 0), stop=(j == FCH - 1))
                ysb = yout.tile([128, LQ], F32, tag="y")
                nc.vector.tensor_scalar_add(ysb, psy, bo_sb[:, fb:fb + 1])
                nc.sync.dma_start(yT.ap()[ts(fb, 128), :], ysb)

    nc.compile()
    return nc


def _tables():
    """RoPE tables in halves-permuted basis: rows i (even-half) hold +sin,
    rows 32+i (odd-half) hold -sin (for the tmp-then-swap formulation)."""
    inv_freq = 1.0 / (10000.0 ** (np.arange(0, D, 2, dtype=np.float64) / D))
    ang = np.outer(inv_freq, np.arange(L, dtype=np.float64))  # [32, L]
    cos = np.cos(ang).astype(np.float32)
    sin = np.sin(ang).astype(np.float32)
    cos64 = np.concatenate([cos, cos], axis=0)                # [64, L]
    sin_sgn = np.concatenate([sin, -sin], axis=0)             # [64, L]
    return cos64, sin_sgn


def _prep_weights(Wq, bq, Wk, bk, Wv, bv, Wo, bo):
    perm = np.concatenate([np.arange(0, D, 2), np.arange(1, D, 2)])
    WqP = np.asarray(Wq, dtype=np.float32)[:, :, perm].reshape(F, H * D)
    bqP = np.asarray(bq, dtype=np.float32)[:, perm].reshape(H * D)
    WkP = np.asarray(Wk, dtype=np.float32)[:, perm]
    bkP = np.asarray(bk, dtype=np.float32)[perm]
    Wkv = np.concatenate([WkP, np.asarray(Wv, dtype=np.float32)], axis=1)
    bkv = np.concatenate([bkP, np.asarray(bv, dtype=np.float32)])
    WoR = np.asarray(Wo, dtype=np.float32).reshape(H * D, F)
    bo_ = np.asarray(bo, dtype=np.float32)

    wq_pret = np.ascontiguousarray(
        WqP.reshape(FCH, 128, FCH, 128).transpose(2, 1, 0, 3)).astype(
            ml_dtypes.bfloat16)
    wkv_pret = np.ascontiguousarray(
        Wkv.reshape(FCH, 128, 128).transpose(1, 0, 2)).astype(
            ml_dtypes.bfloat16)
    wo_pret = np.ascontiguousarray(
        WoR.reshape(FCH, 128, FCH, 128).transpose(2, 1, 0, 3)).astype(
            ml_dtypes.bfloat16)
    bqbo = np.ascontiguousarray(np.concatenate(
        [bqP.reshape(FCH, 128).T, bo_.reshape(FCH, 128).T], axis=1))
    return wq_pret, wkv_pret, wo_pret, bqbo, bkv


def kernel(inputs_q, inputs_kv, mask, Wq, bq, Wk, bk, Wv, bv, Wo, bo):
    if "nc" not in _CACHED:
        _CACHED["nc"] = build_nc()
    nc = _CACHED["nc"]

    wq_pret, wkv_pret, wo_pret, bqbo, bkv = _prep_weights(
        Wq, bq, Wk, bk, Wv, bv, Wo, bo)

    cos64, sin_sgn = _tables()
    scale = 1.0 / np.sqrt(np.float32(D))
    cksk = np.ascontiguousarray(
        np.concatenate([cos64, sin_sgn], axis=1))      # [64, 2*L] (L=LK)
    cosq_full = np.tile(cos64 * scale, (2, 1))         # [128, L]
    sinq_full = np.tile(sin_sgn * scale, (2, 1))

    xq = np.asarray(inputs_q, dtype=np.float32)
    xkv = np.asarray(inputs_kv, dtype=np.float32)
    mk = np.asarray(mask)

    in_maps = []
    for core in range(NCORES):
        b = core // 4
        qs = (core % 4) * LQ
        xq_t = np.ascontiguousarray(
            xq[b, qs:qs + LQ, :].T.reshape(FCH, 128, LQ)).astype(
                ml_dtypes.bfloat16)
        xkv_t = np.ascontiguousarray(
            xkv[b].T.reshape(FCH, 128, NL, LQ).transpose(2, 0, 1, 3)).astype(
                ml_dtypes.bfloat16)
        mask_t = np.ascontiguousarray(
            mk[b, 0, qs:qs + LQ, :].T.reshape(KCH, 128, LQ)
            .astype(np.float16))
        in_maps.append({
            "xq_t": xq_t,
            "xkv_t": xkv_t,
            "mask_t": mask_t,
            "wq": wq_pret,
            "wkv": wkv_pret,
            "wo": wo_pret,
            "bqbo": bqbo,
            "bkv": bkv,
            "cosq": np.ascontiguousarray(
                cosq_full[:, qs:qs + LQ]).astype(ml_dtypes.bfloat16),
            "sinq": np.ascontiguousarray(
                sinq_full[:, qs:qs + LQ]).astype(ml_dtypes.bfloat16),
            "cksk": cksk.astype(ml_dtypes.bfloat16),
        })

    res = bass_utils.run_bass_kernel_spmd(nc, in_maps,
                                          core_ids=list(range(NCORES)))
    _CACHED["last_results"] = res
    _CACHED["last_maps"] = in_maps

    out = np.empty((B, L, F), dtype=np.float32)
    for core in range(NCORES):
        b = core // 4
        qs = (core % 4) * LQ
        out[b, qs:qs + LQ, :] = res.results[core]["yT"].T
    return out
